# revision 6
# baseline (speedup 1.0000x reference)
"""Multi-head attention (B=2, T=2048, D=2048, H=16) on 8 TRN2 NeuronCores.

Tensor-parallel over heads: each core computes 2 heads (dl=256 of D) of the
Q/K/V projections, its heads' attention, and a partial output projection
(columns of Wo). Host sums the 8 partial outputs (the "all-reduce").

Key compaction: keys fully masked by key_padding_mask contribute exact
zeros to every softmax (multiplicative mask), so the host gathers the
unpadded keys per batch and the kernel only processes those (padded up to
an even number of 128-row tiles).

Per-core dataflow (bf16 compute, f32 PSUM accumulation):
  QT_h = (Wq_h/sqrt(dh)) @ q^T        [dh=128, BT=4096]  (transposed layout)
  KT_h = Wk_h @ kC^T                  [dh, ncols]         (compacted keys)
  V    = vC @ Wv_i^T                  [ncols, 256]        (natural layout)
  scoresT = KT_h-chunk.T @ QT_h       [k-tile 128, q 512] per (b, h)
  attnT = exp(scoresT) * maskT        (no max subtraction: |scores| < ~8)
  denom = ones.T @ attnT              (PE column-sum over k, f32 PSUM)
  O^T_h = (V_h-chunk.T @ attnT) * (1/denom)
  partial = O^T.T @ Wo_i^T            [BT, D] -> host sum over cores

The emission schedule keeps the PE continuously busy (TRN2 halves the PE
clock for ~3us after every idle period): b1 projection matmuls fill the
exp/mask wait slots inside b0's attention groups, out-projection units
fill b1's attention groups, and attention for b0's first two q-chunks is
interleaved into the (DMA-paced) tail of b0's projection phase. Mask DMAs
issue from the idle GpSimd queue; wq and the first q chunk are split into
fine-grained tiles so the first matmul starts as early as possible.
"""
import sys

if "/opt/trn_rl_repo" not in sys.path:
    sys.path.insert(0, "/opt/trn_rl_repo")

from collections import deque

import numpy as np
import ml_dtypes

import concourse.bacc as bacc
import concourse.tile as tile
import concourse.mybir as mybir
from concourse import bass_utils

BF16 = ml_dtypes.bfloat16
FP32 = mybir.dt.float32
BF = mybir.dt.bfloat16

B, T, D, H = 2, 2048, 2048, 16
DH = 128
N_CORES = 8
HL = H // N_CORES          # heads per core = 2
DL = HL * DH               # local out dim = 256
BT = B * T                 # 4096
KC = D // 128              # 16 contraction chunks
NBT = BT // 512            # 8 global bt chunks
NQ = T // 512              # 4 q chunks per batch
ND = D // 512              # 4 D chunks

_CACHE = {}


def _kv_chunks(ncols):
    offs, widths = [], []
    o = 0
    while o < ncols:
        w = min(512, ncols - o)
        offs.append(o)
        widths.append(w)
        o += w
    return list(zip(offs, widths))


def _build(nktts):
    # nktts = per-batch compacted k tile counts (128 rows each)
    NKTT = max(nktts)
    ncolsb = [n * 128 for n in nktts]
    chunksb = [_kv_chunks(nc_) for nc_ in ncolsb]
    cumcols = [0]
    for n in ncolsb:
        cumcols.append(cumcols[-1] + n)
    vbase = [0]
    for n in nktts:
        vbase.append(vbase[-1] + n)
    nc = bacc.Bacc("TRN2", target_bir_lowering=False, debug=False,
                   num_devices=N_CORES)
    # chunk-major transposed q: [c, p, kc*512+j]
    qTc = nc.dram_tensor("qTc", [NBT, 128, KC * 512], BF,
                         kind="ExternalInput").ap()
    # compacted transposed k/v: flat concat of per-(b,chunk) blocks
    # [128, KC, w] (p-major)
    kvtot = 128 * KC * cumcols[-1]
    kTc = nc.dram_tensor("kTc", [kvtot], BF, kind="ExternalInput").ap()
    vTc = nc.dram_tensor("vTc", [kvtot], BF, kind="ExternalInput").ap()
    wq = nc.dram_tensor("wq", [128, KC * DL], BF, kind="ExternalInput").ap()
    wk = nc.dram_tensor("wk", [128, KC * DL], BF, kind="ExternalInput").ap()
    wv = nc.dram_tensor("wv", [128, KC * DL], BF, kind="ExternalInput").ap()
    wo = nc.dram_tensor("wo", [128, HL * D], BF, kind="ExternalInput").ap()
    # tiled multiplicative mask over compacted keys, flat per-batch:
    # [(b), qc, kt, p, j]
    mtot = NQ * 128 * 512 * sum(nktts)
    maskt = nc.dram_tensor("maskt", [mtot], BF, kind="ExternalInput").ap()
    mbase = [NQ * 128 * 512 * v for v in vbase]
    out = nc.dram_tensor("out", [BT, D], BF, kind="ExternalOutput").ap()

    EXP = mybir.ActivationFunctionType.Exp
    MUL = mybir.AluOpType.mult

    with tile.TileContext(nc) as tc:
        with tc.tile_pool(name="wpool", bufs=1) as wpool, \
             tc.tile_pool(name="persist", bufs=1) as ppool, \
             tc.tile_pool(name="stream", bufs=8) as spool, \
             tc.tile_pool(name="mask", bufs=sum(nktts) + 1) as mpool, \
             tc.tile_pool(name="attn", bufs=NKTT + 3) as apool, \
             tc.tile_pool(name="small", bufs=2) as rpool, \
             tc.tile_pool(name="ostage", bufs=3) as opool, \
             tc.tile_pool(name="psbig", bufs=3, space="PSUM") as psbig, \
             tc.tile_pool(name="psacc", bufs=1, space="PSUM") as psacc, \
             tc.tile_pool(name="psden", bufs=1, space="PSUM") as psden:

            # ---- weights + constants ----
            # wq split into 4 kc-blocks so the first Q matmul only waits
            # for a 256KB transfer (head-latency), not the full 1MB.
            WB = 4                       # kc-blocks per weight
            KCB = KC // WB               # kc per block = 4
            wq_sb = [wpool.tile([128, KCB * DL], BF, tag=f"wq{i}",
                                name=f"wq{i}")
                     for i in range(WB)]
            wk_sb = wpool.tile([128, KC * DL], BF, tag="wk")
            wv_sb = wpool.tile([128, KC * DL], BF, tag="wv")
            wo_sb = wpool.tile([128, HL * D], BF, tag="wo")

            def wq_col(kc, m):
                return wq_sb[kc // KCB][
                    :, (kc % KCB) * DL + m * 128:(kc % KCB) * DL + (m + 1) * 128]

            ones = wpool.tile([128, 1], BF, tag="ones")

            # ---- persistent activations ----
            QT = [[ppool.tile([128, 512], BF, tag=f"QT{h}_{c}", name=f"QT{h}_{c}")
                   for c in range(NBT)] for h in range(HL)]
            KT = [[ppool.tile([128, ncolsb[b]], BF, tag=f"KT{h}_{b}",
                              name=f"KT{h}_{b}")
                   for b in range(B)] for h in range(HL)]
            OT = [[ppool.tile([128, 512], BF, tag=f"OT{h}_{c}", name=f"OT{h}_{c}")
                   for c in range(NBT)] for h in range(HL)]
            V = [ppool.tile([128, DL], BF, tag=f"V{t}", name=f"V{t}")
                 for t in range(sum(nktts))]

            # ---- DMA issue helpers (input staging) ----
            chunk_q = {}    # (b, cc) -> list of (tile, kc0, nkc)
            chunk_kv = {}   # (nm, b, cc) -> list of (tile, kc0, nkc), width w

            def issue_q(b, cc, parts=2):
                c = b * NQ + cc
                lst = []
                nkc = KC // parts
                for p in range(parts):
                    ch = spool.tile([128, KC // 2, 512], BF, tag="pin",
                                    name=f"pin_q{b}_{cc}_{p}")
                    nc.sync.dma_start(
                        ch[:, :nkc, :],
                        qTc[c, :, p * nkc * 512:(p + 1) * nkc * 512].rearrange(
                            "p (kc j) -> p kc j", j=512))
                    lst.append((ch, p * nkc, nkc))
                chunk_q[b, cc] = lst

            def issue_kv(b, cc):
                j0, w = chunksb[b][cc]
                base = (cumcols[b] + j0) * 128 * KC
                for nm, srct in (("k", kTc), ("v", vTc)):
                    blk = srct[base:base + 128 * KC * w].rearrange(
                        "(p kc j) -> p kc j", p=128, j=w)
                    lst = []
                    for hf in range(2):
                        ch = spool.tile([128, KC // 2, 512], BF, tag="pin",
                                        name=f"pin_{nm}{b}_{cc}_{hf}")
                        nc.sync.dma_start(
                            ch[:, :, :w],
                            blk[:, hf * (KC // 2):(hf + 1) * (KC // 2), :])
                        lst.append((ch, hf * (KC // 2), KC // 2))
                    chunk_kv[nm, b, cc] = (lst, w)

            def slice_of(lst, kc):
                for ch, kc0, nkc in lst:
                    if kc0 <= kc < kc0 + nkc:
                        return ch[:, kc - kc0, :]
                raise KeyError(kc)

            # ---- compute piece emitters ----
            def q_piece(b, cc, m):
                c = b * NQ + cc
                lst = chunk_q[b, cc]
                ps = psbig.tile([128, 1024], FP32, tag="mm")
                for kc in range(KC):
                    nc.tensor.matmul(ps[:, :512], wq_col(kc, m),
                                     slice_of(lst, kc),
                                     start=kc == 0, stop=kc == KC - 1)
                nc.vector.tensor_copy(QT[m][c][:], ps[:, :512])

            def k_piece(b, cc, m):
                lst, w = chunk_kv["k", b, cc]
                j0, _ = chunksb[b][cc]
                ps2 = psbig.tile([128, 1024], FP32, tag="mm")
                for kc in range(KC):
                    nc.tensor.matmul(
                        ps2[:, :w],
                        wk_sb[:, kc * DL + m * 128:kc * DL + (m + 1) * 128],
                        slice_of(lst, kc)[:, :w],
                        start=kc == 0, stop=kc == KC - 1)
                nc.vector.tensor_copy(KT[m][b][:, j0:j0 + w], ps2[:, :w])

            def v_piece(b, cc, sub):
                lst, w = chunk_kv["v", b, cc]
                j0, _ = chunksb[b][cc]
                t = vbase[b] + j0 // 128 + sub
                psv = psbig.tile([128, 1024], FP32, tag="mm", name="psv")
                for kc in range(KC):
                    nc.tensor.matmul(
                        psv[:, :DL],
                        slice_of(lst, kc)[:, sub * 128:(sub + 1) * 128],
                        wv_sb[:, kc * DL:(kc + 1) * DL],
                        start=kc == 0, stop=kc == KC - 1)
                nc.vector.tensor_copy(V[t][:], psv[:, :DL])

            # ---- attention ----
            mtiles = {}     # (b, qc) -> list of mask tiles

            def issue_masks(b, qc):
                nktt = nktts[b]
                tiles = []
                for kt in range(nktt):
                    off = mbase[b] + (qc * nktt + kt) * 128 * 512
                    mt = mpool.tile([128, 512], BF, tag="mask")
                    nc.gpsimd.dma_start(
                        mt[:],
                        maskt[off:off + 128 * 512].rearrange(
                            "(p j) -> p j", p=128))
                    tiles.append(mt)
                mtiles[b, qc] = tiles

            atiles = {}     # (b, qc, h) -> list of attn pair tiles

            def emit_scores(b, qc, h):
                nktt = nktts[b]
                c = b * NQ + qc
                tl = []
                for kp in range((nktt + 1) // 2):
                    nkt = min(2, nktt - kp * 2)   # 1 for odd trailing tile
                    wdt = nkt * 512
                    ps_s = psbig.tile([128, 1024], FP32, tag="mm")
                    for i in range(nkt):
                        kt = kp * 2 + i
                        nc.tensor.matmul(
                            ps_s[:, i * 512:(i + 1) * 512],
                            KT[h][b][:, kt * 128:(kt + 1) * 128],
                            QT[h][c][:],
                            start=True, stop=True)
                    at = apool.tile([128, 1024], BF, tag="attn")
                    nc.scalar.activation(at[:, :wdt], ps_s[:, :wdt], EXP)
                    for i in range(nkt):
                        kt = kp * 2 + i
                        nc.vector.tensor_tensor(
                            at[:, i * 512:(i + 1) * 512],
                            at[:, i * 512:(i + 1) * 512],
                            mtiles[b, qc][kt][:], op=MUL)
                    tl.append(at)
                atiles[b, qc, h] = tl

            def emit_tail(b, qc, h):
                nktt = nktts[b]
                c = b * NQ + qc
                tl = atiles.pop((b, qc, h))
                ps_d = psden.tile([1, 512], FP32, tag="den")
                for kt in range(nktt):
                    nc.tensor.matmul(
                        ps_d[:], ones[:],
                        tl[kt // 2][:, (kt % 2) * 512:(kt % 2 + 1) * 512],
                        start=kt == 0, stop=kt == nktt - 1)
                rec = rpool.tile([1, 512], FP32, tag="rec")
                nc.vector.reciprocal_approx_fast(rec[:], ps_d[:])
                rbc = rpool.tile([128, 512], FP32, tag="rbc")
                nc.gpsimd.partition_broadcast(rbc[:], rec[:])
                ps_o = psacc.tile([128, 512], FP32, tag="acc")
                for kt in range(nktt):
                    nc.tensor.matmul(
                        ps_o[:],
                        V[vbase[b] + kt][:, h * 128:(h + 1) * 128],
                        tl[kt // 2][:, (kt % 2) * 512:(kt % 2 + 1) * 512],
                        start=kt == 0, stop=kt == nktt - 1)
                nc.vector.scalar_tensor_tensor(
                    OT[h][c][:], ps_o[:], 1.0, rbc[:], op0=MUL, op1=MUL)

            # ---- output projection (one 128-row tile per unit) ----
            def outproj_unit(b, qc, sub):
                c = b * NQ + qc
                t = c * 4 + sub
                stage = opool.tile([128, D], BF, tag="ostage")
                for dp in range(2):
                    ps = psbig.tile([128, 1024], FP32, tag="mm")
                    for i in range(2):
                        dc = dp * 2 + i
                        for h in range(HL):
                            nc.tensor.matmul(
                                ps[:, i * 512:(i + 1) * 512],
                                OT[h][c][:, sub * 128:(sub + 1) * 128],
                                wo_sb[:, h * D + dc * 512:h * D + (dc + 1) * 512],
                                start=h == 0, stop=h == HL - 1)
                    if dp == 0:
                        nc.scalar.copy(stage[:, :1024], ps[:])
                    else:
                        nc.vector.tensor_copy(stage[:, 1024:], ps[:])
                nc.sync.dma_start(out[t * 128:(t + 1) * 128, :], stage[:])

            # =========================================================
            # Emission schedule
            # =========================================================
            # --- head: fine-grained first transfers ---
            nc.sync.dma_start(wq_sb[0][:], wq[:, :KCB * DL])
            issue_q(0, 0, parts=4)
            for i in range(1, WB):
                nc.sync.dma_start(wq_sb[i][:],
                                  wq[:, i * KCB * DL:(i + 1) * KCB * DL])
            nc.vector.memset(ones[:], 1.0)
            nc.sync.dma_start(wk_sb[:], wk[:])
            issue_kv(0, 0)
            nc.sync.dma_start(wv_sb[:], wv[:])

            # --- phase A: b0 proj, attention for qc0/qc1 interleaved ---
            q_piece(0, 0, 0)
            issue_q(0, 1)
            q_piece(0, 0, 1)
            issue_kv(0, 1)
            k_piece(0, 0, 0)
            k_piece(0, 0, 1)
            for sub in range(4):
                v_piece(0, 0, sub)
            issue_q(0, 2)
            issue_masks(0, 0)
            q_piece(0, 1, 0)
            q_piece(0, 1, 1)
            k_piece(0, 1, 0)
            k_piece(0, 1, 1)
            for sub in range(4):
                v_piece(0, 1, sub)
            issue_q(0, 3)
            issue_masks(0, 1)
            nc.gpsimd.dma_start(wo_sb[:], wo[:])
            q_piece(0, 2, 0)
            q_piece(0, 2, 1)
            # KT/V of b0 complete; attention can start for qc0/qc1
            emit_scores(0, 0, 0)
            q_piece(0, 3, 0)
            emit_scores(0, 0, 1)
            emit_tail(0, 0, 0)
            q_piece(0, 3, 1)
            emit_scores(0, 1, 0)
            emit_tail(0, 0, 1)

            # --- phase B: b0 attention qc1..3 with b1-proj fillers ---
            # Filler stream: b1 projection pieces interleaved per-cc, with
            # DMA-issue markers ("iq"/"ikv", zero fill-cost) placed one cc
            # ahead of their consumers.
            issue_q(1, 0)
            issue_kv(1, 0)
            fillers = deque()
            for cc in range(NQ):
                fillers.append(("q", 1, cc, 0))
                if cc + 1 < NQ:
                    fillers.append(("iq", 1, cc + 1))
                if cc + 1 < len(chunksb[1]):
                    fillers.append(("ikv", 1, cc + 1))
                fillers.append(("q", 1, cc, 1))
                if cc < len(chunksb[1]):
                    fillers.append(("k", 1, cc, 0))
                    fillers.append(("k", 1, cc, 1))
                    _, w = chunksb[1][cc]
                    for sub in range(w // 128):
                        fillers.append(("v", 1, cc, sub))

            def run_filler(f):
                kind = f[0]
                if kind == "q":
                    q_piece(*f[1:])
                elif kind == "k":
                    k_piece(*f[1:])
                elif kind == "v":
                    v_piece(*f[1:])
                elif kind == "iq":
                    issue_q(*f[1:])
                elif kind == "ikv":
                    issue_kv(*f[1:])
                return kind in ("q", "k", "v")

            def pop_fillers(k):
                done = 0
                while done < k and fillers:
                    if run_filler(fillers.popleft()):
                        done += 1

            # mask prefetch one qc ahead, triggered on h==1 groups
            groups_b0 = [(qc, h) for qc in range(1, NQ) for h in range(HL)][1:]
            prev = (0, 1, 0)  # (b, qc, h) of last un-tailed scores
            for qc, h in groups_b0:
                emit_scores(0, qc, h)
                pop_fillers(3)
                emit_tail(*prev)
                prev = (0, qc, h)
                if h == 1:
                    if qc + 1 < NQ:
                        issue_masks(0, qc + 1)
                    else:
                        issue_masks(1, 0)
            while fillers:
                pop_fillers(1)
            emit_tail(*prev)

            # --- phase C: b1 attention with outproj fillers ---
            ofill = deque()
            for qc in range(NQ):
                for sub in range(4):
                    ofill.append((0, qc, sub))
            # b1 outproj units become ready progressively; mix them in so
            # the final phase is short.
            ready_b1 = deque()

            def pop_units(k):
                for _ in range(k):
                    if ready_b1:
                        outproj_unit(*ready_b1.popleft())
                    elif ofill:
                        outproj_unit(*ofill.popleft())

            prev = None
            for qc in range(NQ):
                for h in range(HL):
                    emit_scores(1, qc, h)
                    pop_units(2)
                    if prev is not None:
                        emit_tail(*prev)
                        if prev[2] == 1:
                            # prev qc's OT now complete for both heads
                            for sub in range(4):
                                ready_b1.append((1, prev[1], sub))
                    prev = (1, qc, h)
                    if h == 1 and qc + 1 < NQ:
                        issue_masks(1, qc + 1)
                pop_units(1)
            emit_tail(*prev)
            for sub in range(4):
                ready_b1.append((1, NQ - 1, sub))

            # --- phase D: drain remaining outproj units ---
            while ofill:
                outproj_unit(*ofill.popleft())
            while ready_b1:
                outproj_unit(*ready_b1.popleft())

    nc.compile()
    return nc


def get_nc(nktts):
    key = tuple(nktts)
    if key not in _CACHE:
        _CACHE[key] = _build(key)
    return _CACHE[key]


def make_in_maps(q, k, v, Wq, Wk, Wv, Wo, attn_mask, key_padding_mask):
    scale = np.float32(1.0 / np.sqrt(np.float32(DH)))

    qT = q.reshape(BT, D).T.astype(BF16)
    qTc = np.ascontiguousarray(
        qT.reshape(KC, 128, NBT, 512).transpose(2, 1, 0, 3)
        .reshape(NBT, 128, KC * 512))

    # ---- compact the key dimension: drop fully-padded keys ----
    kpm = np.asarray(key_padding_mask)
    idx = [np.nonzero(~kpm[b])[0] for b in range(B)]
    nktts = [max(2, -(-len(ix) // 128)) for ix in idx]
    ncolsb = [n * 128 for n in nktts]
    kC = [np.zeros((nc_, D), np.float32) for nc_ in ncolsb]
    vC = [np.zeros((nc_, D), np.float32) for nc_ in ncolsb]
    for b in range(B):
        kC[b][:len(idx[b])] = k[b, idx[b]]
        vC[b][:len(idx[b])] = v[b, idx[b]]

    def flat_chunks(xC):
        blocks = []
        for b in range(B):
            ncols = ncolsb[b]
            xT = xC[b].T.astype(BF16)  # [D, ncols]
            a = xT.reshape(KC, 128, ncols)
            for j0, w in _kv_chunks(ncols):
                # block [128, KC, w], p-major
                blocks.append(np.ascontiguousarray(
                    a[:, :, j0:j0 + w].transpose(1, 0, 2)).reshape(-1))
        return np.ascontiguousarray(np.concatenate(blocks))

    kTc = flat_chunks(kC)
    vTc = flat_chunks(vC)

    # compacted multiplicative transposed mask, flat [(b), qc, kt, p, j]
    mblocks = []
    for b in range(B):
        ncols = ncolsb[b]
        mCb = np.zeros((ncols, T), np.float32)
        mCb[:len(idx[b])] = (~attn_mask[:, idx[b]].T).astype(np.float32)
        mtb = (mCb.astype(BF16)
               .reshape(nktts[b], 128, NQ, 512)
               .transpose(2, 0, 1, 3))      # [NQ, kt, 128, 512]
        mblocks.append(np.ascontiguousarray(mtb).reshape(-1))
    maskt = np.ascontiguousarray(np.concatenate(mblocks))

    def prep_w(wT):  # [D, DL] -> [128, KC*DL]
        return np.ascontiguousarray(
            wT.reshape(KC, 128, DL).transpose(1, 0, 2).reshape(128, KC * DL)
            .astype(BF16))

    in_maps = []
    for i in range(N_CORES):
        rows = slice(i * DL, (i + 1) * DL)
        wq_i = prep_w(Wq[rows, :].T * scale)
        wk_i = prep_w(Wk[rows, :].T)
        wv_i = prep_w(Wv[rows, :].T)
        woT = Wo[:, rows].T  # [DL, D]
        wo_i = np.ascontiguousarray(
            woT.reshape(HL, 128, D).transpose(1, 0, 2).reshape(128, HL * D)
            .astype(BF16))
        in_maps.append({
            "qTc": qTc, "kTc": kTc, "vTc": vTc,
            "wq": wq_i, "wk": wk_i, "wv": wv_i, "wo": wo_i,
            "maskt": maskt,
        })
    return in_maps, nktts


def postprocess(results):
    acc = np.zeros((BT, D), np.float32)
    for r in results:
        acc += r["out"].astype(np.float32)
    return acc.reshape(B, T, D)


def kernel(**inputs):
    inputs = {k: np.asarray(v) for k, v in inputs.items()}
    in_maps, nktts = make_in_maps(**inputs)
    nc = get_nc(nktts)
    res = bass_utils.run_bass_kernel_spmd(
        nc, in_maps, core_ids=list(range(N_CORES)))
    return postprocess(res.results)


# revision 9
# speedup vs baseline: 1.0288x; 1.0288x over previous
"""Multi-head attention (B=2, T=2048, D=2048, H=16) on 8 TRN2 NeuronCores.

Tensor-parallel over heads: each core computes 2 heads (dl=256 of D) of the
Q/K/V projections, its heads' attention, and a partial output projection
(columns of Wo). Host sums the 8 partial outputs (the "all-reduce").

Key compaction: keys fully masked by key_padding_mask contribute exact
zeros to every softmax (multiplicative mask), so the host gathers the
unpadded keys per batch and the kernel only processes those (padded up to
an even number of 128-row tiles).

Per-core dataflow (bf16 compute, f32 PSUM accumulation):
  QT_h = (Wq_h/sqrt(dh)) @ q^T        [dh=128, BT=4096]  (transposed layout)
  KT_h = Wk_h @ kC^T                  [dh, ncols]         (compacted keys)
  V    = vC @ Wv_i^T                  [ncols, 256]        (natural layout)
  scoresT = KT_h-chunk.T @ QT_h       [k-tile 128, q 512] per (b, h)
  attnT = exp(scoresT) * maskT        (no max subtraction: |scores| < ~8)
  denom = ones.T @ attnT              (PE column-sum over k, f32 PSUM)
  O^T_h = (V_h-chunk.T @ attnT) * (1/denom)
  partial = O^T.T @ Wo_i^T            [BT, D] -> host sum over cores

The emission schedule keeps the PE continuously busy (TRN2 halves the PE
clock for ~3us after every idle period): b1 projection matmuls fill the
exp/mask wait slots inside b0's attention groups, out-projection units
fill b1's attention groups, and attention for b0's first two q-chunks is
interleaved into the (DMA-paced) tail of b0's projection phase. Mask DMAs
issue from the idle GpSimd queue; wq and the first q chunk are split into
fine-grained tiles so the first matmul starts as early as possible.
"""
import sys

if "/opt/trn_rl_repo" not in sys.path:
    sys.path.insert(0, "/opt/trn_rl_repo")

from collections import deque

import numpy as np
import ml_dtypes

import concourse.bacc as bacc
import concourse.tile as tile
import concourse.mybir as mybir
from concourse import bass_utils

BF16 = ml_dtypes.bfloat16
FP32 = mybir.dt.float32
BF = mybir.dt.bfloat16

B, T, D, H = 2, 2048, 2048, 16
DH = 128
N_CORES = 8
HL = H // N_CORES          # heads per core = 2
DL = HL * DH               # local out dim = 256
BT = B * T                 # 4096
KC = D // 128              # 16 contraction chunks
NBT = BT // 512            # 8 global bt chunks
NQ = T // 512              # 4 q chunks per batch
ND = D // 512              # 4 D chunks

_CACHE = {}


def _kv_chunks(ncols):
    offs, widths = [], []
    o = 0
    while o < ncols:
        w = min(512, ncols - o)
        offs.append(o)
        widths.append(w)
        o += w
    return list(zip(offs, widths))


def _build(nktts):
    # nktts = per-batch compacted k tile counts (128 rows each)
    NKTT = max(nktts)
    ncolsb = [n * 128 for n in nktts]
    chunksb = [_kv_chunks(nc_) for nc_ in ncolsb]
    cumcols = [0]
    for n in ncolsb:
        cumcols.append(cumcols[-1] + n)
    vbase = [0]
    for n in nktts:
        vbase.append(vbase[-1] + n)
    nc = bacc.Bacc("TRN2", target_bir_lowering=False, debug=False,
                   num_devices=N_CORES)
    # chunk-major transposed q: [c, p, kc*512+j]
    qTc = nc.dram_tensor("qTc", [NBT, 128, KC * 512], BF,
                         kind="ExternalInput").ap()
    # compacted transposed k/v: flat concat of per-(b,chunk) blocks
    # [128, KC, w] (p-major)
    kvtot = 128 * KC * cumcols[-1]
    kTc = nc.dram_tensor("kTc", [kvtot], BF, kind="ExternalInput").ap()
    vTc = nc.dram_tensor("vTc", [kvtot], BF, kind="ExternalInput").ap()
    wq = nc.dram_tensor("wq", [128, KC * DL], BF, kind="ExternalInput").ap()
    wk = nc.dram_tensor("wk", [128, KC * DL], BF, kind="ExternalInput").ap()
    wv = nc.dram_tensor("wv", [128, KC * DL], BF, kind="ExternalInput").ap()
    wo = nc.dram_tensor("wo", [128, HL * D], BF, kind="ExternalInput").ap()
    # tiled multiplicative mask over compacted keys, flat per-batch:
    # [(b), qc, kt, p, j]
    mtot = NQ * 128 * 512 * sum(nktts)
    maskt = nc.dram_tensor("maskt", [mtot], BF, kind="ExternalInput").ap()
    mbase = [NQ * 128 * 512 * v for v in vbase]
    out = nc.dram_tensor("out", [BT, D], BF, kind="ExternalOutput").ap()

    EXP = mybir.ActivationFunctionType.Exp
    MUL = mybir.AluOpType.mult

    with tile.TileContext(nc) as tc:
        with tc.tile_pool(name="wpool", bufs=1) as wpool, \
             tc.tile_pool(name="persist", bufs=1) as ppool, \
             tc.tile_pool(name="stream", bufs=8) as spool, \
             tc.tile_pool(name="mask", bufs=sum(nktts) + 1) as mpool, \
             tc.tile_pool(name="attn", bufs=NKTT + 3) as apool, \
             tc.tile_pool(name="small", bufs=2) as rpool, \
             tc.tile_pool(name="ostage", bufs=3) as opool, \
             tc.tile_pool(name="psbig", bufs=3, space="PSUM") as psbig, \
             tc.tile_pool(name="psacc", bufs=1, space="PSUM") as psacc, \
             tc.tile_pool(name="psden", bufs=1, space="PSUM") as psden:

            # ---- weights + constants ----
            # wq split into 4 kc-blocks so the first Q matmul only waits
            # for a 256KB transfer (head-latency), not the full 1MB.
            WB = 4                       # kc-blocks per weight
            KCB = KC // WB               # kc per block = 4
            wq_sb = [wpool.tile([128, KCB * DL], BF, tag=f"wq{i}",
                                name=f"wq{i}")
                     for i in range(WB)]
            wk_sb = wpool.tile([128, KC * DL], BF, tag="wk")
            wv_sb = wpool.tile([128, KC * DL], BF, tag="wv")
            wo_sb = wpool.tile([128, HL * D], BF, tag="wo")

            def wq_col(kc, m):
                return wq_sb[kc // KCB][
                    :, (kc % KCB) * DL + m * 128:(kc % KCB) * DL + (m + 1) * 128]

            ones = wpool.tile([128, 1], BF, tag="ones")

            # ---- persistent activations ----
            QT = [[ppool.tile([128, 512], BF, tag=f"QT{h}_{c}", name=f"QT{h}_{c}")
                   for c in range(NBT)] for h in range(HL)]
            KT = [[ppool.tile([128, ncolsb[b]], BF, tag=f"KT{h}_{b}",
                              name=f"KT{h}_{b}")
                   for b in range(B)] for h in range(HL)]
            OT = [[ppool.tile([128, 512], BF, tag=f"OT{h}_{c}", name=f"OT{h}_{c}")
                   for c in range(NBT)] for h in range(HL)]
            V = [ppool.tile([128, DL], BF, tag=f"V{t}", name=f"V{t}")
                 for t in range(sum(nktts))]

            # ---- DMA issue helpers (input staging) ----
            chunk_q = {}    # (b, cc) -> list of (tile, kc0, nkc)
            chunk_kv = {}   # (nm, b, cc) -> list of (tile, kc0, nkc), width w

            def issue_q(b, cc, parts=2):
                c = b * NQ + cc
                lst = []
                nkc = KC // parts
                for p in range(parts):
                    ch = spool.tile([128, KC // 2, 512], BF, tag="pin",
                                    name=f"pin_q{b}_{cc}_{p}")
                    nc.sync.dma_start(
                        ch[:, :nkc, :],
                        qTc[c, :, p * nkc * 512:(p + 1) * nkc * 512].rearrange(
                            "p (kc j) -> p kc j", j=512))
                    lst.append((ch, p * nkc, nkc))
                chunk_q[b, cc] = lst

            def issue_kv(b, cc):
                j0, w = chunksb[b][cc]
                base = (cumcols[b] + j0) * 128 * KC
                for nm, srct in (("k", kTc), ("v", vTc)):
                    blk = srct[base:base + 128 * KC * w].rearrange(
                        "(p kc j) -> p kc j", p=128, j=w)
                    lst = []
                    for hf in range(2):
                        ch = spool.tile([128, KC // 2, 512], BF, tag="pin",
                                        name=f"pin_{nm}{b}_{cc}_{hf}")
                        nc.sync.dma_start(
                            ch[:, :, :w],
                            blk[:, hf * (KC // 2):(hf + 1) * (KC // 2), :])
                        lst.append((ch, hf * (KC // 2), KC // 2))
                    chunk_kv[nm, b, cc] = (lst, w)

            def slice_of(lst, kc):
                for ch, kc0, nkc in lst:
                    if kc0 <= kc < kc0 + nkc:
                        return ch[:, kc - kc0, :]
                raise KeyError(kc)

            # ---- compute piece emitters ----
            def q_piece(b, cc, m):
                c = b * NQ + cc
                lst = chunk_q[b, cc]
                ps = psbig.tile([128, 1024], FP32, tag="mm")
                for kc in range(KC):
                    nc.tensor.matmul(ps[:, :512], wq_col(kc, m),
                                     slice_of(lst, kc),
                                     start=kc == 0, stop=kc == KC - 1)
                nc.vector.tensor_copy(QT[m][c][:], ps[:, :512])

            def k_piece(b, cc, m):
                lst, w = chunk_kv["k", b, cc]
                j0, _ = chunksb[b][cc]
                ps2 = psbig.tile([128, 1024], FP32, tag="mm")
                for kc in range(KC):
                    nc.tensor.matmul(
                        ps2[:, :w],
                        wk_sb[:, kc * DL + m * 128:kc * DL + (m + 1) * 128],
                        slice_of(lst, kc)[:, :w],
                        start=kc == 0, stop=kc == KC - 1)
                nc.vector.tensor_copy(KT[m][b][:, j0:j0 + w], ps2[:, :w])

            def v_piece(b, cc, sub):
                lst, w = chunk_kv["v", b, cc]
                j0, _ = chunksb[b][cc]
                t = vbase[b] + j0 // 128 + sub
                psv = psbig.tile([128, 1024], FP32, tag="mm", name="psv")
                for kc in range(KC):
                    nc.tensor.matmul(
                        psv[:, :DL],
                        slice_of(lst, kc)[:, sub * 128:(sub + 1) * 128],
                        wv_sb[:, kc * DL:(kc + 1) * DL],
                        start=kc == 0, stop=kc == KC - 1)
                nc.vector.tensor_copy(V[t][:], psv[:, :DL])

            # ---- attention ----
            mtiles = {}     # (b, qc) -> list of mask tiles

            def issue_masks(b, qc):
                nktt = nktts[b]
                tiles = []
                for kt in range(nktt):
                    off = mbase[b] + (qc * nktt + kt) * 128 * 512
                    mt = mpool.tile([128, 512], BF, tag="mask")
                    nc.gpsimd.dma_start(
                        mt[:],
                        maskt[off:off + 128 * 512].rearrange(
                            "(p j) -> p j", p=128))
                    tiles.append(mt)
                mtiles[b, qc] = tiles

            atiles = {}     # (b, qc, h) -> list of attn pair tiles

            def emit_scores(b, qc, h):
                nktt = nktts[b]
                c = b * NQ + qc
                tl = []
                for kp in range((nktt + 1) // 2):
                    nkt = min(2, nktt - kp * 2)   # 1 for odd trailing tile
                    wdt = nkt * 512
                    ps_s = psbig.tile([128, 1024], FP32, tag="mm")
                    for i in range(nkt):
                        kt = kp * 2 + i
                        nc.tensor.matmul(
                            ps_s[:, i * 512:(i + 1) * 512],
                            KT[h][b][:, kt * 128:(kt + 1) * 128],
                            QT[h][c][:],
                            start=True, stop=True)
                    at = apool.tile([128, 1024], BF, tag="attn")
                    nc.scalar.activation(at[:, :wdt], ps_s[:, :wdt], EXP)
                    for i in range(nkt):
                        kt = kp * 2 + i
                        nc.vector.tensor_tensor(
                            at[:, i * 512:(i + 1) * 512],
                            at[:, i * 512:(i + 1) * 512],
                            mtiles[b, qc][kt][:], op=MUL)
                    tl.append(at)
                atiles[b, qc, h] = tl

            def emit_tail(b, qc, h):
                nktt = nktts[b]
                c = b * NQ + qc
                tl = atiles.pop((b, qc, h))
                ps_d = psden.tile([1, 512], FP32, tag="den")
                for kt in range(nktt):
                    nc.tensor.matmul(
                        ps_d[:], ones[:],
                        tl[kt // 2][:, (kt % 2) * 512:(kt % 2 + 1) * 512],
                        start=kt == 0, stop=kt == nktt - 1)
                rec = rpool.tile([1, 512], FP32, tag="rec")
                nc.vector.reciprocal_approx_fast(rec[:], ps_d[:])
                rbc = rpool.tile([128, 512], FP32, tag="rbc")
                nc.gpsimd.partition_broadcast(rbc[:], rec[:])
                ps_o = psacc.tile([128, 512], FP32, tag="acc")
                for kt in range(nktt):
                    nc.tensor.matmul(
                        ps_o[:],
                        V[vbase[b] + kt][:, h * 128:(h + 1) * 128],
                        tl[kt // 2][:, (kt % 2) * 512:(kt % 2 + 1) * 512],
                        start=kt == 0, stop=kt == nktt - 1)
                nc.vector.scalar_tensor_tensor(
                    OT[h][c][:], ps_o[:], 1.0, rbc[:], op0=MUL, op1=MUL)

            # ---- output projection (one 128-row tile per unit) ----
            def outproj_unit(b, qc, sub):
                c = b * NQ + qc
                t = c * 4 + sub
                stage = opool.tile([128, D], BF, tag="ostage")
                for dp in range(2):
                    ps = psbig.tile([128, 1024], FP32, tag="mm")
                    for i in range(2):
                        dc = dp * 2 + i
                        for h in range(HL):
                            nc.tensor.matmul(
                                ps[:, i * 512:(i + 1) * 512],
                                OT[h][c][:, sub * 128:(sub + 1) * 128],
                                wo_sb[:, h * D + dc * 512:h * D + (dc + 1) * 512],
                                start=h == 0, stop=h == HL - 1)
                    if dp == 0:
                        nc.scalar.copy(stage[:, :1024], ps[:])
                    else:
                        nc.vector.tensor_copy(stage[:, 1024:], ps[:])
                nc.sync.dma_start(out[t * 128:(t + 1) * 128, :], stage[:])

            # =========================================================
            # Emission schedule
            # =========================================================
            # --- head + phase A: b0 proj, DMA issued in exact
            # consumption order (the phase is transfer-bound: ~19MB must
            # land; any out-of-order byte delays the PE) ---
            nc.sync.dma_start(wq_sb[0][:], wq[:, :KCB * DL])
            issue_q(0, 0, parts=4)
            for i in range(1, WB):
                nc.sync.dma_start(wq_sb[i][:],
                                  wq[:, i * KCB * DL:(i + 1) * KCB * DL])
            nc.vector.memset(ones[:], 1.0)
            nc.sync.dma_start(wk_sb[:], wk[:])
            q_piece(0, 0, 0)
            issue_kv(0, 0)
            nc.sync.dma_start(wv_sb[:], wv[:])
            q_piece(0, 0, 1)
            issue_q(0, 1)
            k_piece(0, 0, 0)
            k_piece(0, 0, 1)
            for sub in range(4):
                v_piece(0, 0, sub)
            issue_kv(0, 1)
            q_piece(0, 1, 0)
            q_piece(0, 1, 1)
            issue_q(0, 2)
            k_piece(0, 1, 0)
            k_piece(0, 1, 1)
            for sub in range(4):
                v_piece(0, 1, sub)
            issue_q(0, 3)
            issue_masks(0, 0)
            q_piece(0, 2, 0)
            q_piece(0, 2, 1)
            issue_masks(0, 1)
            # KT/V of b0 complete; attention can start for qc0
            emit_scores(0, 0, 0)
            q_piece(0, 3, 0)
            emit_scores(0, 0, 1)
            emit_tail(0, 0, 0)
            q_piece(0, 3, 1)

            # --- phase B: b0 attention qc1..3 with b1-proj fillers ---
            # Filler stream: b1 projection pieces interleaved per-cc, with
            # DMA-issue markers ("iq"/"ikv", zero fill-cost) placed one cc
            # ahead of their consumers.
            issue_q(1, 0)
            issue_kv(1, 0)
            nc.sync.dma_start(wo_sb[:], wo[:])
            fillers = deque()
            for cc in range(NQ):
                fillers.append(("q", 1, cc, 0))
                if cc + 1 < NQ:
                    fillers.append(("iq", 1, cc + 1))
                if cc + 1 < len(chunksb[1]):
                    fillers.append(("ikv", 1, cc + 1))
                fillers.append(("q", 1, cc, 1))
                if cc < len(chunksb[1]):
                    fillers.append(("k", 1, cc, 0))
                    fillers.append(("k", 1, cc, 1))
                    _, w = chunksb[1][cc]
                    for sub in range(w // 128):
                        fillers.append(("v", 1, cc, sub))

            def run_filler(f):
                kind = f[0]
                if kind == "q":
                    q_piece(*f[1:])
                elif kind == "k":
                    k_piece(*f[1:])
                elif kind == "v":
                    v_piece(*f[1:])
                elif kind == "iq":
                    issue_q(*f[1:])
                elif kind == "ikv":
                    issue_kv(*f[1:])
                return kind in ("q", "k", "v")

            def pop_fillers(k):
                done = 0
                while done < k and fillers:
                    if run_filler(fillers.popleft()):
                        done += 1

            # mask prefetch one qc ahead, triggered on h==1 groups
            groups_b0 = [(qc, h) for qc in range(1, NQ) for h in range(HL)]
            prev = (0, 0, 1)  # (b, qc, h) of last un-tailed scores
            for qc, h in groups_b0:
                emit_scores(0, qc, h)
                pop_fillers(2)
                emit_tail(*prev)
                prev = (0, qc, h)
                if h == 1:
                    if qc + 1 < NQ:
                        issue_masks(0, qc + 1)
                    else:
                        issue_masks(1, 0)
            while fillers:
                pop_fillers(1)
            emit_tail(*prev)

            # --- phase C: b1 attention with outproj fillers ---
            ofill = deque()
            for qc in range(NQ):
                for sub in range(4):
                    ofill.append((0, qc, sub))
            # b1 outproj units become ready progressively; mix them in so
            # the final phase is short.
            ready_b1 = deque()

            def pop_units(k):
                for _ in range(k):
                    if ready_b1:
                        outproj_unit(*ready_b1.popleft())
                    elif ofill:
                        outproj_unit(*ofill.popleft())

            prev = None
            for qc in range(NQ):
                for h in range(HL):
                    emit_scores(1, qc, h)
                    pop_units(2)
                    if prev is not None:
                        emit_tail(*prev)
                        if prev[2] == 1:
                            # prev qc's OT now complete for both heads
                            for sub in range(4):
                                ready_b1.append((1, prev[1], sub))
                    prev = (1, qc, h)
                    if h == 1 and qc + 1 < NQ:
                        issue_masks(1, qc + 1)
                pop_units(1)
            emit_tail(*prev)
            for sub in range(4):
                ready_b1.append((1, NQ - 1, sub))

            # --- phase D: drain remaining outproj units ---
            while ofill:
                outproj_unit(*ofill.popleft())
            while ready_b1:
                outproj_unit(*ready_b1.popleft())

    nc.compile()
    return nc


def get_nc(nktts):
    key = tuple(nktts)
    if key not in _CACHE:
        _CACHE[key] = _build(key)
    return _CACHE[key]


def make_in_maps(q, k, v, Wq, Wk, Wv, Wo, attn_mask, key_padding_mask):
    scale = np.float32(1.0 / np.sqrt(np.float32(DH)))

    qT = q.reshape(BT, D).T.astype(BF16)
    qTc = np.ascontiguousarray(
        qT.reshape(KC, 128, NBT, 512).transpose(2, 1, 0, 3)
        .reshape(NBT, 128, KC * 512))

    # ---- compact the key dimension: drop fully-padded keys ----
    kpm = np.asarray(key_padding_mask)
    idx = [np.nonzero(~kpm[b])[0] for b in range(B)]
    nktts = [max(2, -(-len(ix) // 128)) for ix in idx]
    ncolsb = [n * 128 for n in nktts]
    kC = [np.zeros((nc_, D), np.float32) for nc_ in ncolsb]
    vC = [np.zeros((nc_, D), np.float32) for nc_ in ncolsb]
    for b in range(B):
        kC[b][:len(idx[b])] = k[b, idx[b]]
        vC[b][:len(idx[b])] = v[b, idx[b]]

    def flat_chunks(xC):
        blocks = []
        for b in range(B):
            ncols = ncolsb[b]
            xT = xC[b].T.astype(BF16)  # [D, ncols]
            a = xT.reshape(KC, 128, ncols)
            for j0, w in _kv_chunks(ncols):
                # block [128, KC, w], p-major
                blocks.append(np.ascontiguousarray(
                    a[:, :, j0:j0 + w].transpose(1, 0, 2)).reshape(-1))
        return np.ascontiguousarray(np.concatenate(blocks))

    kTc = flat_chunks(kC)
    vTc = flat_chunks(vC)

    # compacted multiplicative transposed mask, flat [(b), qc, kt, p, j]
    mblocks = []
    for b in range(B):
        ncols = ncolsb[b]
        mCb = np.zeros((ncols, T), np.float32)
        mCb[:len(idx[b])] = (~attn_mask[:, idx[b]].T).astype(np.float32)
        mtb = (mCb.astype(BF16)
               .reshape(nktts[b], 128, NQ, 512)
               .transpose(2, 0, 1, 3))      # [NQ, kt, 128, 512]
        mblocks.append(np.ascontiguousarray(mtb).reshape(-1))
    maskt = np.ascontiguousarray(np.concatenate(mblocks))

    def prep_w(wT):  # [D, DL] -> [128, KC*DL]
        return np.ascontiguousarray(
            wT.reshape(KC, 128, DL).transpose(1, 0, 2).reshape(128, KC * DL)
            .astype(BF16))

    in_maps = []
    for i in range(N_CORES):
        rows = slice(i * DL, (i + 1) * DL)
        wq_i = prep_w(Wq[rows, :].T * scale)
        wk_i = prep_w(Wk[rows, :].T)
        wv_i = prep_w(Wv[rows, :].T)
        woT = Wo[:, rows].T  # [DL, D]
        wo_i = np.ascontiguousarray(
            woT.reshape(HL, 128, D).transpose(1, 0, 2).reshape(128, HL * D)
            .astype(BF16))
        in_maps.append({
            "qTc": qTc, "kTc": kTc, "vTc": vTc,
            "wq": wq_i, "wk": wk_i, "wv": wv_i, "wo": wo_i,
            "maskt": maskt,
        })
    return in_maps, nktts


def postprocess(results):
    acc = np.zeros((BT, D), np.float32)
    for r in results:
        acc += r["out"].astype(np.float32)
    return acc.reshape(B, T, D)


def kernel(**inputs):
    inputs = {k: np.asarray(v) for k, v in inputs.items()}
    in_maps, nktts = make_in_maps(**inputs)
    nc = get_nc(nktts)
    res = bass_utils.run_bass_kernel_spmd(
        nc, in_maps, core_ids=list(range(N_CORES)))
    return postprocess(res.results)


# revision 14
# speedup vs baseline: 1.0792x; 1.0490x over previous
"""Multi-head attention (B=2, T=2048, D=2048, H=16) on 8 TRN2 NeuronCores.

Tensor-parallel over heads: each core computes 2 heads (dl=256 of D) of the
Q/K/V projections, its heads' attention, and a partial output projection
(columns of Wo). Host sums the 8 partial outputs (the "all-reduce").

Key compaction: keys fully masked by key_padding_mask contribute exact
zeros to every softmax (multiplicative mask), so the host gathers the
unpadded keys per batch and the kernel only processes those (padded up to
an even number of 128-row tiles).

Per-core dataflow (bf16 compute, f32 PSUM accumulation):
  QT_h = (Wq_h/sqrt(dh)) @ q^T        [dh=128, BT=4096]  (transposed layout)
  KT_h = Wk_h @ kC^T                  [dh, ncols]         (compacted keys)
  V    = vC @ Wv_i^T                  [ncols, 256]        (natural layout)
  scoresT = KT_h-chunk.T @ QT_h       [k-tile 128, q 512] per (b, h)
  attnT = exp(scoresT) * maskT        (no max subtraction: |scores| < ~8)
  denom = ones.T @ attnT              (PE column-sum over k, f32 PSUM)
  O^T_h = (V_h-chunk.T @ attnT) * (1/denom)
  partial = O^T.T @ Wo_i^T            [BT, D] -> host sum over cores

The emission schedule keeps the PE continuously busy (TRN2 halves the PE
clock for ~3us after every idle period): b1 projection matmuls fill the
exp/mask wait slots inside b0's attention groups, out-projection units
fill b1's attention groups, and attention for b0's first two q-chunks is
interleaved into the (DMA-paced) tail of b0's projection phase. Mask DMAs
issue from the idle GpSimd queue; wq and the first q chunk are split into
fine-grained tiles so the first matmul starts as early as possible.
"""
import sys

if "/opt/trn_rl_repo" not in sys.path:
    sys.path.insert(0, "/opt/trn_rl_repo")

from collections import deque

import numpy as np
import ml_dtypes

import concourse.bacc as bacc
import concourse.tile as tile
import concourse.mybir as mybir
from concourse import bass_utils

BF16 = ml_dtypes.bfloat16
FP32 = mybir.dt.float32
BF = mybir.dt.bfloat16

B, T, D, H = 2, 2048, 2048, 16
DH = 128
N_CORES = 8
HL = H // N_CORES          # heads per core = 2
DL = HL * DH               # local out dim = 256
BT = B * T                 # 4096
KC = D // 128              # 16 contraction chunks
NBT = BT // 512            # 8 global bt chunks
NQ = T // 512              # 4 q chunks per batch
ND = D // 512              # 4 D chunks

_CACHE = {}


def _kv_chunks(ncols):
    offs, widths = [], []
    o = 0
    while o < ncols:
        w = min(512, ncols - o)
        offs.append(o)
        widths.append(w)
        o += w
    return list(zip(offs, widths))


def _build(nktts):
    # nktts = per-batch compacted k tile counts (128 rows each)
    NKTT = max(nktts)
    ncolsb = [n * 128 for n in nktts]
    chunksb = [_kv_chunks(nc_) for nc_ in ncolsb]
    cumcols = [0]
    for n in ncolsb:
        cumcols.append(cumcols[-1] + n)
    vbase = [0]
    for n in nktts:
        vbase.append(vbase[-1] + n)
    nc = bacc.Bacc("TRN2", target_bir_lowering=False, debug=False,
                   num_devices=N_CORES)
    # chunk-major transposed q: [c, p, kc*512+j]
    qTc = nc.dram_tensor("qTc", [NBT, 128, KC * 512], BF,
                         kind="ExternalInput").ap()
    # compacted transposed k/v: flat concat of per-(b,chunk) blocks
    # [128, KC, w] (p-major)
    kvtot = 128 * KC * cumcols[-1]
    kTc = nc.dram_tensor("kTc", [kvtot], BF, kind="ExternalInput").ap()
    vTc = nc.dram_tensor("vTc", [kvtot], BF, kind="ExternalInput").ap()
    wq = nc.dram_tensor("wq", [128, KC * DL], BF, kind="ExternalInput").ap()
    wk = nc.dram_tensor("wk", [128, KC * DL], BF, kind="ExternalInput").ap()
    wv = nc.dram_tensor("wv", [128, KC * DL], BF, kind="ExternalInput").ap()
    wo = nc.dram_tensor("wo", [128, HL * D], BF, kind="ExternalInput").ap()
    # tiled multiplicative mask over compacted keys, flat per-batch:
    # [(b), qc, kt, p, j]
    mtot = NQ * 128 * 512 * sum(nktts)
    maskt = nc.dram_tensor("maskt", [mtot], BF, kind="ExternalInput").ap()
    mbase = [NQ * 128 * 512 * v for v in vbase]
    out = nc.dram_tensor("out", [BT, D], BF, kind="ExternalOutput").ap()

    EXP = mybir.ActivationFunctionType.Exp
    MUL = mybir.AluOpType.mult

    with tile.TileContext(nc) as tc:
        with tc.tile_pool(name="wpool", bufs=1) as wpool, \
             tc.tile_pool(name="persist", bufs=1) as ppool, \
             tc.tile_pool(name="stream", bufs=7) as spool, \
             tc.tile_pool(name="mask", bufs=sum(nktts) + 1) as mpool, \
             tc.tile_pool(name="attn", bufs=NKTT + 6) as apool, \
             tc.tile_pool(name="small", bufs=2) as rpool, \
             tc.tile_pool(name="ostage", bufs=3) as opool, \
             tc.tile_pool(name="psbig", bufs=3, space="PSUM") as psbig, \
             tc.tile_pool(name="psacc", bufs=1, space="PSUM") as psacc, \
             tc.tile_pool(name="psden", bufs=1, space="PSUM") as psden:

            # ---- weights + constants ----
            # wq split into 4 kc-blocks so the first Q matmul only waits
            # for a 256KB transfer (head-latency), not the full 1MB.
            WB = 4                       # kc-blocks per weight
            KCB = KC // WB               # kc per block = 4
            wq_sb = [wpool.tile([128, KCB * DL], BF, tag=f"wq{i}",
                                name=f"wq{i}")
                     for i in range(WB)]
            wk_sb = wpool.tile([128, KC * DL], BF, tag="wk")
            wv_sb = wpool.tile([128, KC * DL], BF, tag="wv")
            wo_sb = wpool.tile([128, HL * D], BF, tag="wo")

            def wq_col(kc, m):
                return wq_sb[kc // KCB][
                    :, (kc % KCB) * DL + m * 128:(kc % KCB) * DL + (m + 1) * 128]

            ones = wpool.tile([128, 1], BF, tag="ones")

            # ---- persistent activations ----
            QT = [[ppool.tile([128, 512], BF, tag=f"QT{h}_{c}", name=f"QT{h}_{c}")
                   for c in range(NBT)] for h in range(HL)]
            KT = [[ppool.tile([128, ncolsb[b]], BF, tag=f"KT{h}_{b}",
                              name=f"KT{h}_{b}")
                   for b in range(B)] for h in range(HL)]
            OT = [[ppool.tile([128, 512], BF, tag=f"OT{h}_{c}", name=f"OT{h}_{c}")
                   for c in range(NBT)] for h in range(HL)]
            V = [ppool.tile([128, DL], BF, tag=f"V{t}", name=f"V{t}")
                 for t in range(sum(nktts))]

            # ---- DMA issue helpers (input staging) ----
            chunk_q = {}    # (b, cc) -> list of (tile, kc0, nkc)
            chunk_kv = {}   # (nm, b, cc) -> list of (tile, kc0, nkc), width w

            def issue_q(b, cc, parts=2):
                c = b * NQ + cc
                lst = []
                nkc = KC // parts
                for p in range(parts):
                    ch = spool.tile([128, KC // 2, 512], BF, tag="pin",
                                    name=f"pin_q{b}_{cc}_{p}")
                    nc.sync.dma_start(
                        ch[:, :nkc, :],
                        qTc[c, :, p * nkc * 512:(p + 1) * nkc * 512].rearrange(
                            "p (kc j) -> p kc j", j=512))
                    lst.append((ch, p * nkc, nkc))
                chunk_q[b, cc] = lst

            def issue_kv(b, cc):
                j0, w = chunksb[b][cc]
                base = (cumcols[b] + j0) * 128 * KC
                for nm, srct in (("k", kTc), ("v", vTc)):
                    blk = srct[base:base + 128 * KC * w].rearrange(
                        "(p kc j) -> p kc j", p=128, j=w)
                    lst = []
                    for hf in range(2):
                        ch = spool.tile([128, KC // 2, 512], BF, tag="pin",
                                        name=f"pin_{nm}{b}_{cc}_{hf}")
                        nc.sync.dma_start(
                            ch[:, :, :w],
                            blk[:, hf * (KC // 2):(hf + 1) * (KC // 2), :])
                        lst.append((ch, hf * (KC // 2), KC // 2))
                    chunk_kv[nm, b, cc] = (lst, w)

            def slice_of(lst, kc):
                for ch, kc0, nkc in lst:
                    if kc0 <= kc < kc0 + nkc:
                        return ch[:, kc - kc0, :]
                raise KeyError(kc)

            # ---- compute piece emitters ----
            def q_piece(b, cc, m):
                c = b * NQ + cc
                lst = chunk_q[b, cc]
                ps = psbig.tile([128, 1024], FP32, tag="mm")
                for kc in range(KC):
                    nc.tensor.matmul(ps[:, :512], wq_col(kc, m),
                                     slice_of(lst, kc),
                                     start=kc == 0, stop=kc == KC - 1)
                nc.scalar.copy(QT[m][c][:], ps[:, :512])

            def k_piece(b, cc, m):
                lst, w = chunk_kv["k", b, cc]
                j0, _ = chunksb[b][cc]
                ps2 = psbig.tile([128, 1024], FP32, tag="mm")
                for kc in range(KC):
                    nc.tensor.matmul(
                        ps2[:, :w],
                        wk_sb[:, kc * DL + m * 128:kc * DL + (m + 1) * 128],
                        slice_of(lst, kc)[:, :w],
                        start=kc == 0, stop=kc == KC - 1)
                nc.scalar.copy(KT[m][b][:, j0:j0 + w], ps2[:, :w])

            def v_piece(b, cc, sub):
                lst, w = chunk_kv["v", b, cc]
                j0, _ = chunksb[b][cc]
                t = vbase[b] + j0 // 128 + sub
                psv = psbig.tile([128, 1024], FP32, tag="mm", name="psv")
                for kc in range(KC):
                    nc.tensor.matmul(
                        psv[:, :DL],
                        slice_of(lst, kc)[:, sub * 128:(sub + 1) * 128],
                        wv_sb[:, kc * DL:(kc + 1) * DL],
                        start=kc == 0, stop=kc == KC - 1)
                nc.scalar.copy(V[t][:], psv[:, :DL])

            # ---- attention ----
            mtiles = {}     # (b, qc) -> list of mask tiles

            def issue_masks(b, qc):
                nktt = nktts[b]
                tiles = []
                for kt in range(nktt):
                    off = mbase[b] + (qc * nktt + kt) * 128 * 512
                    mt = mpool.tile([128, 512], BF, tag="mask")
                    nc.sync.dma_start(
                        mt[:],
                        maskt[off:off + 128 * 512].rearrange(
                            "(p j) -> p j", p=128))
                    tiles.append(mt)
                mtiles[b, qc] = tiles

            atiles = {}     # (b, qc, h) -> list of attn pair tiles

            def emit_scores(b, qc, h):
                nktt = nktts[b]
                c = b * NQ + qc
                tl = []
                for kp in range((nktt + 1) // 2):
                    nkt = min(2, nktt - kp * 2)   # 1 for odd trailing tile
                    wdt = nkt * 512
                    ps_s = psbig.tile([128, 1024], FP32, tag="mm")
                    for i in range(nkt):
                        kt = kp * 2 + i
                        nc.tensor.matmul(
                            ps_s[:, i * 512:(i + 1) * 512],
                            KT[h][b][:, kt * 128:(kt + 1) * 128],
                            QT[h][c][:],
                            start=True, stop=True)
                    at = apool.tile([128, 1024], BF, tag="attn")
                    nc.scalar.activation(at[:, :wdt], ps_s[:, :wdt], EXP)
                    for i in range(nkt):
                        kt = kp * 2 + i
                        nc.vector.tensor_tensor(
                            at[:, i * 512:(i + 1) * 512],
                            at[:, i * 512:(i + 1) * 512],
                            mtiles[b, qc][kt][:], op=MUL)
                    tl.append(at)
                atiles[b, qc, h] = tl

            def emit_tail(b, qc, h):
                nktt = nktts[b]
                c = b * NQ + qc
                tl = atiles.pop((b, qc, h))
                ps_d = psden.tile([1, 512], FP32, tag="den")
                for kt in range(nktt):
                    nc.tensor.matmul(
                        ps_d[:], ones[:],
                        tl[kt // 2][:, (kt % 2) * 512:(kt % 2 + 1) * 512],
                        start=kt == 0, stop=kt == nktt - 1)
                rec = rpool.tile([1, 512], FP32, tag="rec")
                nc.vector.reciprocal_approx_fast(rec[:], ps_d[:])
                rbc = rpool.tile([128, 512], FP32, tag="rbc")
                nc.gpsimd.partition_broadcast(rbc[:], rec[:])
                ps_o = psacc.tile([128, 512], FP32, tag="acc")
                for kt in range(nktt):
                    nc.tensor.matmul(
                        ps_o[:],
                        V[vbase[b] + kt][:, h * 128:(h + 1) * 128],
                        tl[kt // 2][:, (kt % 2) * 512:(kt % 2 + 1) * 512],
                        start=kt == 0, stop=kt == nktt - 1)
                nc.vector.scalar_tensor_tensor(
                    OT[h][c][:], ps_o[:], 1.0, rbc[:], op0=MUL, op1=MUL)

            # ---- output projection (one 128-row tile per unit) ----
            def outproj_unit(b, qc, sub):
                c = b * NQ + qc
                t = c * 4 + sub
                stage = opool.tile([128, D], BF, tag="ostage")
                for dp in range(2):
                    ps = psbig.tile([128, 1024], FP32, tag="mm")
                    for i in range(2):
                        dc = dp * 2 + i
                        for h in range(HL):
                            nc.tensor.matmul(
                                ps[:, i * 512:(i + 1) * 512],
                                OT[h][c][:, sub * 128:(sub + 1) * 128],
                                wo_sb[:, h * D + dc * 512:h * D + (dc + 1) * 512],
                                start=h == 0, stop=h == HL - 1)
                    if dp == 0:
                        nc.scalar.copy(stage[:, :1024], ps[:])
                    else:
                        nc.vector.tensor_copy(stage[:, 1024:], ps[:])
                nc.sync.dma_start(out[t * 128:(t + 1) * 128, :], stage[:])

            # =========================================================
            # Emission schedule
            # =========================================================
            # --- head + phase A: b0 proj, DMA issued in exact
            # consumption order (the phase is transfer-bound: ~19MB must
            # land; any out-of-order byte delays the PE) ---
            nc.sync.dma_start(wq_sb[0][:], wq[:, :KCB * DL])
            issue_q(0, 0, parts=4)
            for i in range(1, WB):
                nc.sync.dma_start(wq_sb[i][:],
                                  wq[:, i * KCB * DL:(i + 1) * KCB * DL])
            nc.vector.memset(ones[:], 1.0)
            nc.sync.dma_start(wk_sb[:], wk[:])
            q_piece(0, 0, 0)
            issue_kv(0, 0)
            nc.sync.dma_start(wv_sb[:], wv[:])
            q_piece(0, 0, 1)
            issue_q(0, 1)
            k_piece(0, 0, 0)
            k_piece(0, 0, 1)
            for sub in range(4):
                v_piece(0, 0, sub)
            issue_kv(0, 1)
            q_piece(0, 1, 0)
            q_piece(0, 1, 1)
            issue_q(0, 2)
            k_piece(0, 1, 0)
            k_piece(0, 1, 1)
            for sub in range(4):
                v_piece(0, 1, sub)
            issue_q(0, 3)
            issue_masks(0, 0)
            q_piece(0, 2, 0)
            q_piece(0, 2, 1)
            issue_masks(0, 1)
            # KT/V of b0 complete; attention can start for qc0
            emit_scores(0, 0, 0)
            q_piece(0, 3, 0)
            emit_scores(0, 0, 1)
            q_piece(0, 3, 1)

            # --- phase B: b0 attention qc1..3 with b1-proj fillers ---
            # Filler stream: b1 projection pieces interleaved per-cc, with
            # DMA-issue markers ("iq"/"ikv", zero fill-cost) placed one cc
            # ahead of their consumers.
            issue_q(1, 0)
            issue_kv(1, 0)
            nc.sync.dma_start(wo_sb[:], wo[:])
            fillers = deque()
            for cc in range(NQ):
                fillers.append(("q", 1, cc, 0))
                if cc + 1 < NQ:
                    fillers.append(("iq", 1, cc + 1))
                if cc + 1 < len(chunksb[1]):
                    fillers.append(("ikv", 1, cc + 1))
                fillers.append(("q", 1, cc, 1))
                if cc < len(chunksb[1]):
                    fillers.append(("k", 1, cc, 0))
                    fillers.append(("k", 1, cc, 1))
                    _, w = chunksb[1][cc]
                    for sub in range(w // 128):
                        fillers.append(("v", 1, cc, sub))

            def run_filler(f):
                kind = f[0]
                if kind == "q":
                    q_piece(*f[1:])
                elif kind == "k":
                    k_piece(*f[1:])
                elif kind == "v":
                    v_piece(*f[1:])
                elif kind == "iq":
                    issue_q(*f[1:])
                elif kind == "ikv":
                    issue_kv(*f[1:])
                return kind in ("q", "k", "v")

            def pop_fillers(k):
                done = 0
                while done < k and fillers:
                    if run_filler(fillers.popleft()):
                        done += 1

            # mask prefetch one qc ahead, triggered on h==1 groups.
            # 2-deep pipeline: tail(g-2) in slot g, so exp+mask of a group
            # have ~2 slots of latency budget before its tail needs them.
            groups_b0 = [(qc, h) for qc in range(1, NQ) for h in range(HL)]
            pending = deque([(0, 0, 0), (0, 0, 1)])
            for qc, h in groups_b0:
                emit_scores(0, qc, h)
                pop_fillers(2)
                emit_tail(*pending.popleft())
                pending.append((0, qc, h))
                if h == 1:
                    if qc + 1 < NQ:
                        issue_masks(0, qc + 1)
                    else:
                        issue_masks(1, 0)
            while fillers:
                pop_fillers(1)
            while pending:
                emit_tail(*pending.popleft())

            # --- phase C: b1 attention with outproj fillers ---
            ofill = deque()
            for qc in range(NQ):
                for sub in range(4):
                    ofill.append((0, qc, sub))
            # b1 outproj units become ready progressively; mix them in so
            # the final phase is short.
            ready_b1 = deque()

            def pop_units(k):
                for _ in range(k):
                    if ready_b1:
                        outproj_unit(*ready_b1.popleft())
                    elif ofill:
                        outproj_unit(*ofill.popleft())

            pend = deque()
            for qc in range(NQ):
                for h in range(HL):
                    emit_scores(1, qc, h)
                    pop_units(2)
                    if len(pend) >= 2:
                        g = pend.popleft()
                        emit_tail(*g)
                        if g[2] == 1:
                            # g's qc now has OT complete for both heads
                            for sub in range(4):
                                ready_b1.append((1, g[1], sub))
                    pend.append((1, qc, h))
                    if h == 1 and qc + 1 < NQ:
                        issue_masks(1, qc + 1)
                pop_units(1)
            while pend:
                g = pend.popleft()
                emit_tail(*g)
                pop_units(2)
            for sub in range(4):
                ready_b1.append((1, NQ - 1, sub))

            # --- phase D: drain remaining outproj units ---
            while ofill:
                outproj_unit(*ofill.popleft())
            while ready_b1:
                outproj_unit(*ready_b1.popleft())

    nc.compile()
    return nc


def get_nc(nktts):
    key = tuple(nktts)
    if key not in _CACHE:
        _CACHE[key] = _build(key)
    return _CACHE[key]


def make_in_maps(q, k, v, Wq, Wk, Wv, Wo, attn_mask, key_padding_mask):
    scale = np.float32(1.0 / np.sqrt(np.float32(DH)))

    qT = q.reshape(BT, D).T.astype(BF16)
    qTc = np.ascontiguousarray(
        qT.reshape(KC, 128, NBT, 512).transpose(2, 1, 0, 3)
        .reshape(NBT, 128, KC * 512))

    # ---- compact the key dimension: drop fully-padded keys ----
    kpm = np.asarray(key_padding_mask)
    idx = [np.nonzero(~kpm[b])[0] for b in range(B)]
    nktts = [max(2, -(-len(ix) // 128)) for ix in idx]
    ncolsb = [n * 128 for n in nktts]
    kC = [np.zeros((nc_, D), np.float32) for nc_ in ncolsb]
    vC = [np.zeros((nc_, D), np.float32) for nc_ in ncolsb]
    for b in range(B):
        kC[b][:len(idx[b])] = k[b, idx[b]]
        vC[b][:len(idx[b])] = v[b, idx[b]]

    def flat_chunks(xC):
        blocks = []
        for b in range(B):
            ncols = ncolsb[b]
            xT = xC[b].T.astype(BF16)  # [D, ncols]
            a = xT.reshape(KC, 128, ncols)
            for j0, w in _kv_chunks(ncols):
                # block [128, KC, w], p-major
                blocks.append(np.ascontiguousarray(
                    a[:, :, j0:j0 + w].transpose(1, 0, 2)).reshape(-1))
        return np.ascontiguousarray(np.concatenate(blocks))

    kTc = flat_chunks(kC)
    vTc = flat_chunks(vC)

    # compacted multiplicative transposed mask, flat [(b), qc, kt, p, j]
    mblocks = []
    for b in range(B):
        ncols = ncolsb[b]
        mCb = np.zeros((ncols, T), np.float32)
        mCb[:len(idx[b])] = (~attn_mask[:, idx[b]].T).astype(np.float32)
        mtb = (mCb.astype(BF16)
               .reshape(nktts[b], 128, NQ, 512)
               .transpose(2, 0, 1, 3))      # [NQ, kt, 128, 512]
        mblocks.append(np.ascontiguousarray(mtb).reshape(-1))
    maskt = np.ascontiguousarray(np.concatenate(mblocks))

    def prep_w(wT):  # [D, DL] -> [128, KC*DL]
        return np.ascontiguousarray(
            wT.reshape(KC, 128, DL).transpose(1, 0, 2).reshape(128, KC * DL)
            .astype(BF16))

    in_maps = []
    for i in range(N_CORES):
        rows = slice(i * DL, (i + 1) * DL)
        wq_i = prep_w(Wq[rows, :].T * scale)
        wk_i = prep_w(Wk[rows, :].T)
        wv_i = prep_w(Wv[rows, :].T)
        woT = Wo[:, rows].T  # [DL, D]
        wo_i = np.ascontiguousarray(
            woT.reshape(HL, 128, D).transpose(1, 0, 2).reshape(128, HL * D)
            .astype(BF16))
        in_maps.append({
            "qTc": qTc, "kTc": kTc, "vTc": vTc,
            "wq": wq_i, "wk": wk_i, "wv": wv_i, "wo": wo_i,
            "maskt": maskt,
        })
    return in_maps, nktts


def postprocess(results):
    acc = np.zeros((BT, D), np.float32)
    for r in results:
        acc += r["out"].astype(np.float32)
    return acc.reshape(B, T, D)


def kernel(**inputs):
    inputs = {k: np.asarray(v) for k, v in inputs.items()}
    in_maps, nktts = make_in_maps(**inputs)
    nc = get_nc(nktts)
    res = bass_utils.run_bass_kernel_spmd(
        nc, in_maps, core_ids=list(range(N_CORES)))
    return postprocess(res.results)


# revision 16
# speedup vs baseline: 1.1083x; 1.0270x over previous
"""Multi-head attention (B=2, T=2048, D=2048, H=16) on 8 TRN2 NeuronCores.

Tensor-parallel over heads: each core computes 2 heads (dl=256 of D) of the
Q/K/V projections, its heads' attention, and a partial output projection
(columns of Wo). Host sums the 8 partial outputs (the "all-reduce").

Key compaction: keys fully masked by key_padding_mask contribute exact
zeros to every softmax (multiplicative mask), so the host gathers the
unpadded keys per batch and the kernel only processes those (padded up to
an even number of 128-row tiles).

Per-core dataflow (bf16 compute, f32 PSUM accumulation):
  QT_h = (Wq_h/sqrt(dh)) @ q^T        [dh=128, BT=4096]  (transposed layout)
  KT_h = Wk_h @ kC^T                  [dh, ncols]         (compacted keys)
  V    = vC @ Wv_i^T                  [ncols, 256]        (natural layout)
  scoresT = KT_h-chunk.T @ QT_h       [k-tile 128, q 512] per (b, h)
  attnT = exp(scoresT) * maskT        (no max subtraction: |scores| < ~8)
  denom = ones.T @ attnT              (PE column-sum over k, f32 PSUM)
  O^T_h = (V_h-chunk.T @ attnT) * (1/denom)
  partial = O^T.T @ Wo_i^T            [BT, D] -> host sum over cores

The emission schedule keeps the PE continuously busy (TRN2 halves the PE
clock for ~3us after every idle period): b1 projection matmuls fill the
exp/mask wait slots inside b0's attention groups, out-projection units
fill b1's attention groups, and attention for b0's first two q-chunks is
interleaved into the (DMA-paced) tail of b0's projection phase. Mask DMAs
issue from the idle GpSimd queue; wq and the first q chunk are split into
fine-grained tiles so the first matmul starts as early as possible.
"""
import sys

if "/opt/trn_rl_repo" not in sys.path:
    sys.path.insert(0, "/opt/trn_rl_repo")

from collections import deque

import numpy as np
import ml_dtypes

import concourse.bacc as bacc
import concourse.tile as tile
import concourse.mybir as mybir
from concourse import bass_utils

BF16 = ml_dtypes.bfloat16
FP32 = mybir.dt.float32
BF = mybir.dt.bfloat16

B, T, D, H = 2, 2048, 2048, 16
DH = 128
N_CORES = 8
HL = H // N_CORES          # heads per core = 2
DL = HL * DH               # local out dim = 256
BT = B * T                 # 4096
KC = D // 128              # 16 contraction chunks
NBT = BT // 512            # 8 global bt chunks
NQ = T // 512              # 4 q chunks per batch
ND = D // 512              # 4 D chunks

_CACHE = {}


def _kv_chunks(ncols):
    offs, widths = [], []
    o = 0
    while o < ncols:
        w = min(512, ncols - o)
        offs.append(o)
        widths.append(w)
        o += w
    return list(zip(offs, widths))


def _build(nktts):
    # nktts = per-batch compacted k tile counts (128 rows each)
    NKTT = max(nktts)
    ncolsb = [n * 128 for n in nktts]
    chunksb = [_kv_chunks(nc_) for nc_ in ncolsb]
    cumcols = [0]
    for n in ncolsb:
        cumcols.append(cumcols[-1] + n)
    vbase = [0]
    for n in nktts:
        vbase.append(vbase[-1] + n)
    nc = bacc.Bacc("TRN2", target_bir_lowering=False, debug=False,
                   num_devices=N_CORES)
    # chunk-major transposed q: [c, p, kc*512+j]
    qTc = nc.dram_tensor("qTc", [NBT, 128, KC * 512], BF,
                         kind="ExternalInput").ap()
    # compacted transposed k/v: flat concat of per-(b,chunk) blocks
    # [128, KC, w] (p-major)
    kvtot = 128 * KC * cumcols[-1]
    kTc = nc.dram_tensor("kTc", [kvtot], BF, kind="ExternalInput").ap()
    vTc = nc.dram_tensor("vTc", [kvtot], BF, kind="ExternalInput").ap()
    wq = nc.dram_tensor("wq", [128, KC * DL], BF, kind="ExternalInput").ap()
    wk = nc.dram_tensor("wk", [128, KC * DL], BF, kind="ExternalInput").ap()
    wv = nc.dram_tensor("wv", [128, KC * DL], BF, kind="ExternalInput").ap()
    wo = nc.dram_tensor("wo", [128, HL * D], BF, kind="ExternalInput").ap()
    # tiled multiplicative mask over compacted keys, flat per-batch:
    # [(b), qc, kt, p, j]
    mtot = NQ * 128 * 512 * sum(nktts)
    maskt = nc.dram_tensor("maskt", [mtot], BF, kind="ExternalInput").ap()
    mbase = [NQ * 128 * 512 * v for v in vbase]
    out = nc.dram_tensor("out", [BT, D], BF, kind="ExternalOutput").ap()

    EXP = mybir.ActivationFunctionType.Exp
    MUL = mybir.AluOpType.mult

    with tile.TileContext(nc) as tc:
        with tc.tile_pool(name="wpool", bufs=1) as wpool, \
             tc.tile_pool(name="persist", bufs=1) as ppool, \
             tc.tile_pool(name="stream", bufs=7) as spool, \
             tc.tile_pool(name="mask", bufs=sum(nktts) + 1) as mpool, \
             tc.tile_pool(name="attn", bufs=NKTT + 6) as apool, \
             tc.tile_pool(name="small", bufs=2) as rpool, \
             tc.tile_pool(name="ostage", bufs=3) as opool, \
             tc.tile_pool(name="psbig", bufs=3, space="PSUM") as psbig, \
             tc.tile_pool(name="psacc", bufs=1, space="PSUM") as psacc, \
             tc.tile_pool(name="psden", bufs=1, space="PSUM") as psden:

            # ---- weights + constants ----
            # wq split into 4 kc-blocks so the first Q matmul only waits
            # for a 256KB transfer (head-latency), not the full 1MB.
            WB = 4                       # kc-blocks per weight
            KCB = KC // WB               # kc per block = 4
            wq_sb = [wpool.tile([128, KCB * DL], BF, tag=f"wq{i}",
                                name=f"wq{i}")
                     for i in range(WB)]
            wk_sb = wpool.tile([128, KC * DL], BF, tag="wk")
            wv_sb = wpool.tile([128, KC * DL], BF, tag="wv")
            wo_sb = wpool.tile([128, HL * D], BF, tag="wo")

            def wq_col(kc, m):
                return wq_sb[kc // KCB][
                    :, (kc % KCB) * DL + m * 128:(kc % KCB) * DL + (m + 1) * 128]

            ones = wpool.tile([128, 1], BF, tag="ones")

            # ---- persistent activations ----
            QT = [[ppool.tile([128, 512], BF, tag=f"QT{h}_{c}", name=f"QT{h}_{c}")
                   for c in range(NBT)] for h in range(HL)]
            KT = [[ppool.tile([128, ncolsb[b]], BF, tag=f"KT{h}_{b}",
                              name=f"KT{h}_{b}")
                   for b in range(B)] for h in range(HL)]
            OT = [[ppool.tile([128, 512], BF, tag=f"OT{h}_{c}", name=f"OT{h}_{c}")
                   for c in range(NBT)] for h in range(HL)]
            V = [ppool.tile([128, DL], BF, tag=f"V{t}", name=f"V{t}")
                 for t in range(sum(nktts))]

            # ---- DMA issue helpers (input staging) ----
            chunk_q = {}    # (b, cc) -> list of (tile, kc0, nkc)
            chunk_kv = {}   # (nm, b, cc) -> list of (tile, kc0, nkc), width w

            def issue_q(b, cc, parts=2):
                c = b * NQ + cc
                lst = []
                nkc = KC // parts
                for p in range(parts):
                    ch = spool.tile([128, KC // 2, 512], BF, tag="pin",
                                    name=f"pin_q{b}_{cc}_{p}")
                    nc.sync.dma_start(
                        ch[:, :nkc, :],
                        qTc[c, :, p * nkc * 512:(p + 1) * nkc * 512].rearrange(
                            "p (kc j) -> p kc j", j=512))
                    lst.append((ch, p * nkc, nkc))
                chunk_q[b, cc] = lst

            def issue_kv(b, cc):
                j0, w = chunksb[b][cc]
                base = (cumcols[b] + j0) * 128 * KC
                for nm, srct in (("k", kTc), ("v", vTc)):
                    blk = srct[base:base + 128 * KC * w].rearrange(
                        "(p kc j) -> p kc j", p=128, j=w)
                    lst = []
                    for hf in range(2):
                        ch = spool.tile([128, KC // 2, 512], BF, tag="pin",
                                        name=f"pin_{nm}{b}_{cc}_{hf}")
                        nc.sync.dma_start(
                            ch[:, :, :w],
                            blk[:, hf * (KC // 2):(hf + 1) * (KC // 2), :])
                        lst.append((ch, hf * (KC // 2), KC // 2))
                    chunk_kv[nm, b, cc] = (lst, w)

            def slice_of(lst, kc):
                for ch, kc0, nkc in lst:
                    if kc0 <= kc < kc0 + nkc:
                        return ch[:, kc - kc0, :]
                raise KeyError(kc)

            # ---- compute piece emitters ----
            def q_piece(b, cc, m):
                c = b * NQ + cc
                lst = chunk_q[b, cc]
                ps = psbig.tile([128, 1024], FP32, tag="mm")
                for kc in range(KC):
                    nc.tensor.matmul(ps[:, :512], wq_col(kc, m),
                                     slice_of(lst, kc),
                                     start=kc == 0, stop=kc == KC - 1)
                nc.scalar.copy(QT[m][c][:], ps[:, :512])

            def k_piece(b, cc, m):
                lst, w = chunk_kv["k", b, cc]
                j0, _ = chunksb[b][cc]
                ps2 = psbig.tile([128, 1024], FP32, tag="mm")
                for kc in range(KC):
                    nc.tensor.matmul(
                        ps2[:, :w],
                        wk_sb[:, kc * DL + m * 128:kc * DL + (m + 1) * 128],
                        slice_of(lst, kc)[:, :w],
                        start=kc == 0, stop=kc == KC - 1)
                nc.scalar.copy(KT[m][b][:, j0:j0 + w], ps2[:, :w])

            def v_piece(b, cc, sub):
                lst, w = chunk_kv["v", b, cc]
                j0, _ = chunksb[b][cc]
                t = vbase[b] + j0 // 128 + sub
                psv = psbig.tile([128, 1024], FP32, tag="mm", name="psv")
                for kc in range(KC):
                    nc.tensor.matmul(
                        psv[:, :DL],
                        slice_of(lst, kc)[:, sub * 128:(sub + 1) * 128],
                        wv_sb[:, kc * DL:(kc + 1) * DL],
                        start=kc == 0, stop=kc == KC - 1)
                nc.scalar.copy(V[t][:], psv[:, :DL])

            # ---- attention ----
            mtiles = {}     # (b, qc) -> list of mask tiles

            def issue_masks(b, qc):
                nktt = nktts[b]
                tiles = []
                for kt in range(nktt):
                    off = mbase[b] + (qc * nktt + kt) * 128 * 512
                    mt = mpool.tile([128, 512], BF, tag="mask")
                    nc.sync.dma_start(
                        mt[:],
                        maskt[off:off + 128 * 512].rearrange(
                            "(p j) -> p j", p=128))
                    tiles.append(mt)
                mtiles[b, qc] = tiles

            atiles = {}     # (b, qc, h) -> list of attn pair tiles

            def emit_scores(b, qc, h, kps=None):
                nktt = nktts[b]
                c = b * NQ + qc
                tl = atiles.setdefault((b, qc, h), [])
                if kps is None:
                    kps = range((nktt + 1) // 2)
                for kp in kps:
                    nkt = min(2, nktt - kp * 2)   # 1 for odd trailing tile
                    wdt = nkt * 512
                    ps_s = psbig.tile([128, 1024], FP32, tag="mm")
                    for i in range(nkt):
                        kt = kp * 2 + i
                        nc.tensor.matmul(
                            ps_s[:, i * 512:(i + 1) * 512],
                            KT[h][b][:, kt * 128:(kt + 1) * 128],
                            QT[h][c][:],
                            start=True, stop=True)
                    at = apool.tile([128, 1024], BF, tag="attn")
                    nc.scalar.activation(at[:, :wdt], ps_s[:, :wdt], EXP)
                    for i in range(nkt):
                        kt = kp * 2 + i
                        nc.vector.tensor_tensor(
                            at[:, i * 512:(i + 1) * 512],
                            at[:, i * 512:(i + 1) * 512],
                            mtiles[b, qc][kt][:], op=MUL)
                    tl.append(at)

            def emit_tail(b, qc, h):
                nktt = nktts[b]
                c = b * NQ + qc
                tl = atiles.pop((b, qc, h))
                ps_d = psden.tile([1, 512], FP32, tag="den")
                for kt in range(nktt):
                    nc.tensor.matmul(
                        ps_d[:], ones[:],
                        tl[kt // 2][:, (kt % 2) * 512:(kt % 2 + 1) * 512],
                        start=kt == 0, stop=kt == nktt - 1)
                rec = rpool.tile([1, 512], FP32, tag="rec")
                nc.vector.reciprocal_approx_fast(rec[:], ps_d[:])
                rbc = rpool.tile([128, 512], FP32, tag="rbc")
                nc.gpsimd.partition_broadcast(rbc[:], rec[:])
                ps_o = psacc.tile([128, 512], FP32, tag="acc")
                for kt in range(nktt):
                    nc.tensor.matmul(
                        ps_o[:],
                        V[vbase[b] + kt][:, h * 128:(h + 1) * 128],
                        tl[kt // 2][:, (kt % 2) * 512:(kt % 2 + 1) * 512],
                        start=kt == 0, stop=kt == nktt - 1)
                nc.vector.scalar_tensor_tensor(
                    OT[h][c][:], ps_o[:], 1.0, rbc[:], op0=MUL, op1=MUL)

            # ---- output projection (one 128-row tile per unit) ----
            def outproj_unit(b, qc, sub):
                c = b * NQ + qc
                t = c * 4 + sub
                stage = opool.tile([128, D], BF, tag="ostage")
                for dp in range(2):
                    ps = psbig.tile([128, 1024], FP32, tag="mm")
                    for i in range(2):
                        dc = dp * 2 + i
                        for h in range(HL):
                            nc.tensor.matmul(
                                ps[:, i * 512:(i + 1) * 512],
                                OT[h][c][:, sub * 128:(sub + 1) * 128],
                                wo_sb[:, h * D + dc * 512:h * D + (dc + 1) * 512],
                                start=h == 0, stop=h == HL - 1)
                    if dp == 0:
                        nc.scalar.copy(stage[:, :1024], ps[:])
                    else:
                        nc.vector.tensor_copy(stage[:, 1024:], ps[:])
                nc.sync.dma_start(out[t * 128:(t + 1) * 128, :], stage[:])

            # =========================================================
            # Emission schedule
            # =========================================================
            # --- head + phase A: b0 proj, DMA issued in exact
            # consumption order (the phase is transfer-bound: ~19MB must
            # land; any out-of-order byte delays the PE) ---
            nc.sync.dma_start(wq_sb[0][:], wq[:, :KCB * DL])
            issue_q(0, 0, parts=4)
            for i in range(1, WB):
                nc.sync.dma_start(wq_sb[i][:],
                                  wq[:, i * KCB * DL:(i + 1) * KCB * DL])
            nc.vector.memset(ones[:], 1.0)
            nc.sync.dma_start(wk_sb[:], wk[:])
            q_piece(0, 0, 0)
            issue_kv(0, 0)
            nc.sync.dma_start(wv_sb[:], wv[:])
            q_piece(0, 0, 1)
            issue_q(0, 1)
            k_piece(0, 0, 0)
            k_piece(0, 0, 1)
            issue_masks(0, 0)
            # qc0 scores over cc0's k-tiles: fills the PE while v00 lands
            # (the scores matmuls need neither masks nor V; the mask-mults
            # on vector wait for the mask DMA without blocking anything)
            emit_scores(0, 0, 0, kps=[0, 1])
            emit_scores(0, 0, 1, kps=[0, 1])
            for sub in range(4):
                v_piece(0, 0, sub)
            issue_kv(0, 1)
            q_piece(0, 1, 0)
            q_piece(0, 1, 1)
            issue_q(0, 2)
            k_piece(0, 1, 0)
            k_piece(0, 1, 1)
            emit_scores(0, 0, 0, kps=[2, 3])
            emit_scores(0, 0, 1, kps=[2, 3])
            for sub in range(4):
                v_piece(0, 1, sub)
            issue_q(0, 3)
            issue_masks(0, 1)
            q_piece(0, 2, 0)
            q_piece(0, 2, 1)
            q_piece(0, 3, 0)
            q_piece(0, 3, 1)

            # --- phase B: b0 attention qc1..3 with b1-proj fillers ---
            # Filler stream: b1 projection pieces interleaved per-cc, with
            # DMA-issue markers ("iq"/"ikv", zero fill-cost) placed one cc
            # ahead of their consumers.
            issue_q(1, 0)
            issue_kv(1, 0)
            nc.sync.dma_start(wo_sb[:], wo[:])
            fillers = deque()
            for cc in range(NQ):
                fillers.append(("q", 1, cc, 0))
                if cc + 1 < NQ:
                    fillers.append(("iq", 1, cc + 1))
                if cc + 1 < len(chunksb[1]):
                    fillers.append(("ikv", 1, cc + 1))
                fillers.append(("q", 1, cc, 1))
                if cc < len(chunksb[1]):
                    fillers.append(("k", 1, cc, 0))
                    fillers.append(("k", 1, cc, 1))
                    _, w = chunksb[1][cc]
                    for sub in range(w // 128):
                        fillers.append(("v", 1, cc, sub))

            def run_filler(f):
                kind = f[0]
                if kind == "q":
                    q_piece(*f[1:])
                elif kind == "k":
                    k_piece(*f[1:])
                elif kind == "v":
                    v_piece(*f[1:])
                elif kind == "iq":
                    issue_q(*f[1:])
                elif kind == "ikv":
                    issue_kv(*f[1:])
                return kind in ("q", "k", "v")

            def pop_fillers(k):
                done = 0
                while done < k and fillers:
                    if run_filler(fillers.popleft()):
                        done += 1

            # mask prefetch one qc ahead, triggered on h==1 groups.
            # 2-deep pipeline: tail(g-2) in slot g, so exp+mask of a group
            # have ~2 slots of latency budget before its tail needs them.
            groups_b0 = [(qc, h) for qc in range(1, NQ) for h in range(HL)]
            pending = deque([(0, 0, 0), (0, 0, 1)])
            for qc, h in groups_b0:
                emit_scores(0, qc, h)
                pop_fillers(2)
                emit_tail(*pending.popleft())
                pending.append((0, qc, h))
                if h == 1:
                    if qc + 1 < NQ:
                        issue_masks(0, qc + 1)
                    else:
                        issue_masks(1, 0)
            while fillers:
                pop_fillers(1)
            while pending:
                emit_tail(*pending.popleft())

            # --- phase C: b1 attention with outproj fillers ---
            ofill = deque()
            for qc in range(NQ):
                for sub in range(4):
                    ofill.append((0, qc, sub))
            # b1 outproj units become ready progressively; mix them in so
            # the final phase is short.
            ready_b1 = deque()

            def pop_units(k):
                for _ in range(k):
                    if ready_b1:
                        outproj_unit(*ready_b1.popleft())
                    elif ofill:
                        outproj_unit(*ofill.popleft())

            pend = deque()
            for qc in range(NQ):
                for h in range(HL):
                    emit_scores(1, qc, h)
                    pop_units(3)
                    if len(pend) >= 2:
                        g = pend.popleft()
                        emit_tail(*g)
                        if g[2] == 1:
                            # g's qc now has OT complete for both heads
                            for sub in range(4):
                                ready_b1.append((1, g[1], sub))
                    pend.append((1, qc, h))
                    if h == 1 and qc + 1 < NQ:
                        issue_masks(1, qc + 1)
                pop_units(1)
            while pend:
                g = pend.popleft()
                emit_tail(*g)
                pop_units(2)
            for sub in range(4):
                ready_b1.append((1, NQ - 1, sub))

            # --- phase D: drain remaining outproj units ---
            while ofill:
                outproj_unit(*ofill.popleft())
            while ready_b1:
                outproj_unit(*ready_b1.popleft())

    nc.compile()
    return nc


def get_nc(nktts):
    key = tuple(nktts)
    if key not in _CACHE:
        _CACHE[key] = _build(key)
    return _CACHE[key]


def make_in_maps(q, k, v, Wq, Wk, Wv, Wo, attn_mask, key_padding_mask):
    scale = np.float32(1.0 / np.sqrt(np.float32(DH)))

    qT = q.reshape(BT, D).T.astype(BF16)
    qTc = np.ascontiguousarray(
        qT.reshape(KC, 128, NBT, 512).transpose(2, 1, 0, 3)
        .reshape(NBT, 128, KC * 512))

    # ---- compact the key dimension: drop fully-padded keys ----
    kpm = np.asarray(key_padding_mask)
    idx = [np.nonzero(~kpm[b])[0] for b in range(B)]
    nktts = [max(2, -(-len(ix) // 128)) for ix in idx]
    ncolsb = [n * 128 for n in nktts]
    kC = [np.zeros((nc_, D), np.float32) for nc_ in ncolsb]
    vC = [np.zeros((nc_, D), np.float32) for nc_ in ncolsb]
    for b in range(B):
        kC[b][:len(idx[b])] = k[b, idx[b]]
        vC[b][:len(idx[b])] = v[b, idx[b]]

    def flat_chunks(xC):
        blocks = []
        for b in range(B):
            ncols = ncolsb[b]
            xT = xC[b].T.astype(BF16)  # [D, ncols]
            a = xT.reshape(KC, 128, ncols)
            for j0, w in _kv_chunks(ncols):
                # block [128, KC, w], p-major
                blocks.append(np.ascontiguousarray(
                    a[:, :, j0:j0 + w].transpose(1, 0, 2)).reshape(-1))
        return np.ascontiguousarray(np.concatenate(blocks))

    kTc = flat_chunks(kC)
    vTc = flat_chunks(vC)

    # compacted multiplicative transposed mask, flat [(b), qc, kt, p, j]
    mblocks = []
    for b in range(B):
        ncols = ncolsb[b]
        mCb = np.zeros((ncols, T), np.float32)
        mCb[:len(idx[b])] = (~attn_mask[:, idx[b]].T).astype(np.float32)
        mtb = (mCb.astype(BF16)
               .reshape(nktts[b], 128, NQ, 512)
               .transpose(2, 0, 1, 3))      # [NQ, kt, 128, 512]
        mblocks.append(np.ascontiguousarray(mtb).reshape(-1))
    maskt = np.ascontiguousarray(np.concatenate(mblocks))

    def prep_w(wT):  # [D, DL] -> [128, KC*DL]
        return np.ascontiguousarray(
            wT.reshape(KC, 128, DL).transpose(1, 0, 2).reshape(128, KC * DL)
            .astype(BF16))

    in_maps = []
    for i in range(N_CORES):
        rows = slice(i * DL, (i + 1) * DL)
        wq_i = prep_w(Wq[rows, :].T * scale)
        wk_i = prep_w(Wk[rows, :].T)
        wv_i = prep_w(Wv[rows, :].T)
        woT = Wo[:, rows].T  # [DL, D]
        wo_i = np.ascontiguousarray(
            woT.reshape(HL, 128, D).transpose(1, 0, 2).reshape(128, HL * D)
            .astype(BF16))
        in_maps.append({
            "qTc": qTc, "kTc": kTc, "vTc": vTc,
            "wq": wq_i, "wk": wk_i, "wv": wv_i, "wo": wo_i,
            "maskt": maskt,
        })
    return in_maps, nktts


def postprocess(results):
    acc = np.zeros((BT, D), np.float32)
    for r in results:
        acc += r["out"].astype(np.float32)
    return acc.reshape(B, T, D)


def kernel(**inputs):
    inputs = {k: np.asarray(v) for k, v in inputs.items()}
    in_maps, nktts = make_in_maps(**inputs)
    nc = get_nc(nktts)
    res = bass_utils.run_bass_kernel_spmd(
        nc, in_maps, core_ids=list(range(N_CORES)))
    return postprocess(res.results)


# revision 17
# speedup vs baseline: 1.1422x; 1.0306x over previous
"""Multi-head attention (B=2, T=2048, D=2048, H=16) on 8 TRN2 NeuronCores.

Tensor-parallel over heads: each core computes 2 heads (dl=256 of D) of the
Q/K/V projections, its heads' attention, and a partial output projection
(columns of Wo). Host sums the 8 partial outputs (the "all-reduce").

Key compaction: keys fully masked by key_padding_mask contribute exact
zeros to every softmax (multiplicative mask), so the host gathers the
unpadded keys per batch and the kernel only processes those (padded up to
an even number of 128-row tiles).

Per-core dataflow (bf16 compute, f32 PSUM accumulation):
  QT_h = (Wq_h/sqrt(dh)) @ q^T        [dh=128, BT=4096]  (transposed layout)
  KT_h = Wk_h @ kC^T                  [dh, ncols]         (compacted keys)
  V    = vC @ Wv_i^T                  [ncols, 256]        (natural layout)
  scoresT = KT_h-chunk.T @ QT_h       [k-tile 128, q 512] per (b, h)
  attnT = exp(scoresT) * maskT        (no max subtraction: |scores| < ~8)
  denom = ones.T @ attnT              (PE column-sum over k, f32 PSUM)
  O^T_h = (V_h-chunk.T @ attnT) * (1/denom)
  partial = O^T.T @ Wo_i^T            [BT, D] -> host sum over cores

The emission schedule keeps the PE continuously busy (TRN2 halves the PE
clock for ~3us after every idle period): b1 projection matmuls fill the
exp/mask wait slots inside b0's attention groups, out-projection units
fill b1's attention groups, and attention for b0's first two q-chunks is
interleaved into the (DMA-paced) tail of b0's projection phase. Mask DMAs
issue from the idle GpSimd queue; wq and the first q chunk are split into
fine-grained tiles so the first matmul starts as early as possible.
"""
import sys

if "/opt/trn_rl_repo" not in sys.path:
    sys.path.insert(0, "/opt/trn_rl_repo")

from collections import deque

import numpy as np
import ml_dtypes

import concourse.bacc as bacc
import concourse.tile as tile
import concourse.mybir as mybir
from concourse import bass_utils

BF16 = ml_dtypes.bfloat16
FP32 = mybir.dt.float32
BF = mybir.dt.bfloat16

B, T, D, H = 2, 2048, 2048, 16
DH = 128
N_CORES = 8
HL = H // N_CORES          # heads per core = 2
DL = HL * DH               # local out dim = 256
BT = B * T                 # 4096
KC = D // 128              # 16 contraction chunks
NBT = BT // 512            # 8 global bt chunks
NQ = T // 512              # 4 q chunks per batch
ND = D // 512              # 4 D chunks

_CACHE = {}


def _kv_chunks(ncols):
    offs, widths = [], []
    o = 0
    while o < ncols:
        w = min(512, ncols - o)
        offs.append(o)
        widths.append(w)
        o += w
    return list(zip(offs, widths))


def _build(nktts):
    # nktts = per-batch compacted k tile counts (128 rows each)
    NKTT = max(nktts)
    ncolsb = [n * 128 for n in nktts]
    chunksb = [_kv_chunks(nc_) for nc_ in ncolsb]
    cumcols = [0]
    for n in ncolsb:
        cumcols.append(cumcols[-1] + n)
    vbase = [0]
    for n in nktts:
        vbase.append(vbase[-1] + n)
    nc = bacc.Bacc("TRN2", target_bir_lowering=False, debug=False,
                   num_devices=N_CORES)
    # chunk-major transposed q: [c, p, kc*512+j]
    qTc = nc.dram_tensor("qTc", [NBT, 128, KC * 512], BF,
                         kind="ExternalInput").ap()
    # compacted transposed k/v: flat concat of per-(b,chunk) blocks
    # [128, KC, w] (p-major)
    kvtot = 128 * KC * cumcols[-1]
    kTc = nc.dram_tensor("kTc", [kvtot], BF, kind="ExternalInput").ap()
    vTc = nc.dram_tensor("vTc", [kvtot], BF, kind="ExternalInput").ap()
    wq = nc.dram_tensor("wq", [128, KC * DL], BF, kind="ExternalInput").ap()
    wk = nc.dram_tensor("wk", [128, KC * DL], BF, kind="ExternalInput").ap()
    wv = nc.dram_tensor("wv", [128, KC * DL], BF, kind="ExternalInput").ap()
    wo = nc.dram_tensor("wo", [128, HL * D], BF, kind="ExternalInput").ap()
    # tiled multiplicative mask over compacted keys, flat per-batch:
    # [(b), qc, kt, p, j]
    mtot = NQ * 128 * 512 * sum(nktts)
    maskt = nc.dram_tensor("maskt", [mtot], BF, kind="ExternalInput").ap()
    mbase = [NQ * 128 * 512 * v for v in vbase]
    out = nc.dram_tensor("out", [BT, D], BF, kind="ExternalOutput").ap()

    EXP = mybir.ActivationFunctionType.Exp
    MUL = mybir.AluOpType.mult

    with tile.TileContext(nc) as tc:
        with tc.tile_pool(name="wpool", bufs=1) as wpool, \
             tc.tile_pool(name="persist", bufs=1) as ppool, \
             tc.tile_pool(name="stream", bufs=7) as spool, \
             tc.tile_pool(name="mask", bufs=sum(nktts)) as mpool, \
             tc.tile_pool(name="attn", bufs=NKTT + 6) as apool, \
             tc.tile_pool(name="dsum", bufs=4) as dpool, \
             tc.tile_pool(name="small", bufs=2) as rpool, \
             tc.tile_pool(name="ostage", bufs=2) as opool, \
             tc.tile_pool(name="psbig", bufs=3, space="PSUM") as psbig, \
             tc.tile_pool(name="psacc", bufs=1, space="PSUM") as psacc, \
             tc.tile_pool(name="psden", bufs=1, space="PSUM") as psden:

            # ---- weights + constants ----
            # wq split into 4 kc-blocks so the first Q matmul only waits
            # for a 256KB transfer (head-latency), not the full 1MB.
            WB = 4                       # kc-blocks per weight
            KCB = KC // WB               # kc per block = 4
            wq_sb = [wpool.tile([128, KCB * DL], BF, tag=f"wq{i}",
                                name=f"wq{i}")
                     for i in range(WB)]
            wk_sb = wpool.tile([128, KC * DL], BF, tag="wk")
            wv_sb = wpool.tile([128, KC * DL], BF, tag="wv")
            wo_sb = wpool.tile([128, HL * D], BF, tag="wo")

            def wq_col(kc, m):
                return wq_sb[kc // KCB][
                    :, (kc % KCB) * DL + m * 128:(kc % KCB) * DL + (m + 1) * 128]

            ones = wpool.tile([128, 1], BF, tag="ones")

            # ---- persistent activations ----
            QT = [[ppool.tile([128, 512], BF, tag=f"QT{h}_{c}", name=f"QT{h}_{c}")
                   for c in range(NBT)] for h in range(HL)]
            KT = [[ppool.tile([128, ncolsb[b]], BF, tag=f"KT{h}_{b}",
                              name=f"KT{h}_{b}")
                   for b in range(B)] for h in range(HL)]
            OT = [[ppool.tile([128, 512], BF, tag=f"OT{h}_{c}", name=f"OT{h}_{c}")
                   for c in range(NBT)] for h in range(HL)]
            V = [ppool.tile([128, DL], BF, tag=f"V{t}", name=f"V{t}")
                 for t in range(sum(nktts))]

            # ---- DMA issue helpers (input staging) ----
            chunk_q = {}    # (b, cc) -> list of (tile, kc0, nkc)
            chunk_kv = {}   # (nm, b, cc) -> list of (tile, kc0, nkc), width w

            def issue_q(b, cc, splits=(8, 8)):
                c = b * NQ + cc
                lst = []
                kc0 = 0
                for p, nkc in enumerate(splits):
                    ch = spool.tile([128, KC // 2, 512], BF, tag="pin",
                                    name=f"pin_q{b}_{cc}_{p}")
                    nc.sync.dma_start(
                        ch[:, :nkc, :],
                        qTc[c, :, kc0 * 512:(kc0 + nkc) * 512].rearrange(
                            "p (kc j) -> p kc j", j=512))
                    lst.append((ch, kc0, nkc))
                    kc0 += nkc
                chunk_q[b, cc] = lst

            def issue_kv(b, cc):
                j0, w = chunksb[b][cc]
                base = (cumcols[b] + j0) * 128 * KC
                for nm, srct in (("k", kTc), ("v", vTc)):
                    blk = srct[base:base + 128 * KC * w].rearrange(
                        "(p kc j) -> p kc j", p=128, j=w)
                    lst = []
                    for hf in range(2):
                        ch = spool.tile([128, KC // 2, 512], BF, tag="pin",
                                        name=f"pin_{nm}{b}_{cc}_{hf}")
                        nc.sync.dma_start(
                            ch[:, :, :w],
                            blk[:, hf * (KC // 2):(hf + 1) * (KC // 2), :])
                        lst.append((ch, hf * (KC // 2), KC // 2))
                    chunk_kv[nm, b, cc] = (lst, w)

            def slice_of(lst, kc):
                for ch, kc0, nkc in lst:
                    if kc0 <= kc < kc0 + nkc:
                        return ch[:, kc - kc0, :]
                raise KeyError(kc)

            # ---- compute piece emitters ----
            def q_piece(b, cc, m):
                c = b * NQ + cc
                lst = chunk_q[b, cc]
                ps = psbig.tile([128, 1024], FP32, tag="mm")
                for kc in range(KC):
                    nc.tensor.matmul(ps[:, :512], wq_col(kc, m),
                                     slice_of(lst, kc),
                                     start=kc == 0, stop=kc == KC - 1)
                nc.scalar.copy(QT[m][c][:], ps[:, :512])

            def k_piece(b, cc, m):
                lst, w = chunk_kv["k", b, cc]
                j0, _ = chunksb[b][cc]
                ps2 = psbig.tile([128, 1024], FP32, tag="mm")
                for kc in range(KC):
                    nc.tensor.matmul(
                        ps2[:, :w],
                        wk_sb[:, kc * DL + m * 128:kc * DL + (m + 1) * 128],
                        slice_of(lst, kc)[:, :w],
                        start=kc == 0, stop=kc == KC - 1)
                nc.scalar.copy(KT[m][b][:, j0:j0 + w], ps2[:, :w])

            def v_piece(b, cc, sub):
                lst, w = chunk_kv["v", b, cc]
                j0, _ = chunksb[b][cc]
                t = vbase[b] + j0 // 128 + sub
                psv = psbig.tile([128, 1024], FP32, tag="mm", name="psv")
                for kc in range(KC):
                    nc.tensor.matmul(
                        psv[:, :DL],
                        slice_of(lst, kc)[:, sub * 128:(sub + 1) * 128],
                        wv_sb[:, kc * DL:(kc + 1) * DL],
                        start=kc == 0, stop=kc == KC - 1)
                nc.scalar.copy(V[t][:], psv[:, :DL])

            # ---- attention ----
            mtiles = {}     # (b, qc) -> list of mask tiles

            def issue_masks(b, qc):
                nktt = nktts[b]
                tiles = []
                for kt in range(nktt):
                    off = mbase[b] + (qc * nktt + kt) * 128 * 512
                    mt = mpool.tile([128, 512], BF, tag="mask")
                    nc.sync.dma_start(
                        mt[:],
                        maskt[off:off + 128 * 512].rearrange(
                            "(p j) -> p j", p=128))
                    tiles.append(mt)
                mtiles[b, qc] = tiles

            atiles = {}     # (b, qc, h) -> list of attn pair tiles

            def emit_scores(b, qc, h, kps=None):
                nktt = nktts[b]
                c = b * NQ + qc
                tl = atiles.setdefault((b, qc, h), [])
                if kps is None:
                    kps = range((nktt + 1) // 2)
                for kp in kps:
                    nkt = min(2, nktt - kp * 2)   # 1 for odd trailing tile
                    wdt = nkt * 512
                    ps_s = psbig.tile([128, 1024], FP32, tag="mm")
                    for i in range(nkt):
                        kt = kp * 2 + i
                        nc.tensor.matmul(
                            ps_s[:, i * 512:(i + 1) * 512],
                            KT[h][b][:, kt * 128:(kt + 1) * 128],
                            QT[h][c][:],
                            start=True, stop=True)
                    at = apool.tile([128, 1024], BF, tag="attn")
                    nc.scalar.activation(at[:, :wdt], ps_s[:, :wdt], EXP)
                    for i in range(nkt):
                        kt = kp * 2 + i
                        nc.vector.tensor_tensor(
                            at[:, i * 512:(i + 1) * 512],
                            at[:, i * 512:(i + 1) * 512],
                            mtiles[b, qc][kt][:], op=MUL)
                    tl.append(at)

            def emit_tail(b, qc, h):
                nktt = nktts[b]
                c = b * NQ + qc
                tl = atiles.pop((b, qc, h))
                ps_d = psden.tile([1, 512], FP32, tag="den")
                if b == 0 and nktt % 2 == 0:
                    # pairwise tree-sum over k-tiles on the vector engine,
                    # then a single ones-matmul: saves 7 PE matmuls/group
                    npair = nktt // 2
                    ds = [dpool.tile([128, 512], BF, tag="ds", name=f"ds{i}")
                          for i in range(npair)]
                    for i in range(npair):
                        nc.vector.tensor_tensor(
                            ds[i][:], tl[i][:, :512], tl[i][:, 512:],
                            op=mybir.AluOpType.add)
                    step = 1
                    while step < npair:
                        for i in range(0, npair, 2 * step):
                            nc.vector.tensor_tensor(
                                ds[i][:], ds[i][:], ds[i + step][:],
                                op=mybir.AluOpType.add)
                        step *= 2
                    nc.tensor.matmul(ps_d[:], ones[:], ds[0][:],
                                     start=True, stop=True)
                else:
                    for kt in range(nktt):
                        nc.tensor.matmul(
                            ps_d[:], ones[:],
                            tl[kt // 2][:, (kt % 2) * 512:(kt % 2 + 1) * 512],
                            start=kt == 0, stop=kt == nktt - 1)
                rec = rpool.tile([1, 512], FP32, tag="rec")
                nc.vector.reciprocal_approx_fast(rec[:], ps_d[:])
                rbc = rpool.tile([128, 512], FP32, tag="rbc")
                nc.gpsimd.partition_broadcast(rbc[:], rec[:])
                ps_o = psacc.tile([128, 512], FP32, tag="acc")
                for kt in range(nktt):
                    nc.tensor.matmul(
                        ps_o[:],
                        V[vbase[b] + kt][:, h * 128:(h + 1) * 128],
                        tl[kt // 2][:, (kt % 2) * 512:(kt % 2 + 1) * 512],
                        start=kt == 0, stop=kt == nktt - 1)
                nc.vector.scalar_tensor_tensor(
                    OT[h][c][:], ps_o[:], 1.0, rbc[:], op0=MUL, op1=MUL)

            # ---- output projection (one 128-row tile per unit) ----
            def outproj_unit(b, qc, sub, fine=False):
                c = b * NQ + qc
                t = c * 4 + sub
                stage = opool.tile([128, D], BF, tag="ostage")
                for dp in range(2):
                    ps = psbig.tile([128, 1024], FP32, tag="mm")
                    for i in range(2):
                        dc = dp * 2 + i
                        for h in range(HL):
                            nc.tensor.matmul(
                                ps[:, i * 512:(i + 1) * 512],
                                OT[h][c][:, sub * 128:(sub + 1) * 128],
                                wo_sb[:, h * D + dc * 512:h * D + (dc + 1) * 512],
                                start=h == 0, stop=h == HL - 1)
                    if dp == 0:
                        nc.scalar.copy(stage[:, :1024], ps[:])
                    else:
                        nc.vector.tensor_copy(stage[:, 1024:], ps[:])
                    if fine:
                        nc.sync.dma_start(
                            out[t * 128:(t + 1) * 128,
                                dp * 1024:(dp + 1) * 1024],
                            stage[:, dp * 1024:(dp + 1) * 1024])
                if not fine:
                    nc.sync.dma_start(out[t * 128:(t + 1) * 128, :], stage[:])

            # =========================================================
            # Emission schedule
            # =========================================================
            # --- head + phase A: b0 proj, DMA issued in exact
            # consumption order (the phase is transfer-bound: ~19MB must
            # land; any out-of-order byte delays the PE) ---
            nc.sync.dma_start(wq_sb[0][:], wq[:, :KCB * DL])
            issue_q(0, 0, splits=(2, 2, 4, 8))
            for i in range(1, WB):
                nc.sync.dma_start(wq_sb[i][:],
                                  wq[:, i * KCB * DL:(i + 1) * KCB * DL])
            nc.vector.memset(ones[:], 1.0)
            nc.sync.dma_start(wk_sb[:], wk[:])
            q_piece(0, 0, 0)
            issue_kv(0, 0)
            nc.sync.dma_start(wv_sb[:], wv[:])
            q_piece(0, 0, 1)
            issue_q(0, 1)
            k_piece(0, 0, 0)
            k_piece(0, 0, 1)
            issue_masks(0, 0)
            # qc0 scores over cc0's k-tiles: fills the PE while v00 lands
            # (the scores matmuls need neither masks nor V; the mask-mults
            # on vector wait for the mask DMA without blocking anything)
            emit_scores(0, 0, 0, kps=[0, 1])
            emit_scores(0, 0, 1, kps=[0, 1])
            for sub in range(4):
                v_piece(0, 0, sub)
            issue_kv(0, 1)
            q_piece(0, 1, 0)
            q_piece(0, 1, 1)
            issue_q(0, 2)
            k_piece(0, 1, 0)
            k_piece(0, 1, 1)
            emit_scores(0, 0, 0, kps=[2, 3])
            emit_scores(0, 0, 1, kps=[2, 3])
            for sub in range(4):
                v_piece(0, 1, sub)
            issue_q(0, 3)
            issue_masks(0, 1)
            q_piece(0, 2, 0)
            q_piece(0, 2, 1)
            q_piece(0, 3, 0)
            q_piece(0, 3, 1)

            # --- phase B: b0 attention qc1..3 with b1-proj fillers ---
            # Filler stream: b1 projection pieces interleaved per-cc, with
            # DMA-issue markers ("iq"/"ikv", zero fill-cost) placed one cc
            # ahead of their consumers.
            issue_q(1, 0)
            issue_kv(1, 0)
            nc.sync.dma_start(wo_sb[:], wo[:])
            fillers = deque()
            for cc in range(NQ):
                fillers.append(("q", 1, cc, 0))
                if cc + 1 < NQ:
                    fillers.append(("iq", 1, cc + 1))
                if cc + 1 < len(chunksb[1]):
                    fillers.append(("ikv", 1, cc + 1))
                fillers.append(("q", 1, cc, 1))
                if cc < len(chunksb[1]):
                    fillers.append(("k", 1, cc, 0))
                    fillers.append(("k", 1, cc, 1))
                    _, w = chunksb[1][cc]
                    for sub in range(w // 128):
                        fillers.append(("v", 1, cc, sub))

            def run_filler(f):
                kind = f[0]
                if kind == "q":
                    q_piece(*f[1:])
                elif kind == "k":
                    k_piece(*f[1:])
                elif kind == "v":
                    v_piece(*f[1:])
                elif kind == "iq":
                    issue_q(*f[1:])
                elif kind == "ikv":
                    issue_kv(*f[1:])
                return kind in ("q", "k", "v")

            def pop_fillers(k):
                done = 0
                while done < k and fillers:
                    if run_filler(fillers.popleft()):
                        done += 1

            # mask prefetch one qc ahead, triggered on h==1 groups.
            # 2-deep pipeline: tail(g-2) in slot g, so exp+mask of a group
            # have ~2 slots of latency budget before its tail needs them.
            groups_b0 = [(qc, h) for qc in range(1, NQ) for h in range(HL)]
            pending = deque([(0, 0, 0), (0, 0, 1)])
            for qc, h in groups_b0:
                emit_scores(0, qc, h)
                pop_fillers(2)
                emit_tail(*pending.popleft())
                pending.append((0, qc, h))
                if h == 1:
                    if qc + 1 < NQ:
                        issue_masks(0, qc + 1)
                    else:
                        issue_masks(1, 0)
            while fillers:
                pop_fillers(1)
            while pending:
                emit_tail(*pending.popleft())

            # --- phase C: b1 attention with outproj fillers ---
            ofill = deque()
            for qc in range(NQ):
                for sub in range(4):
                    ofill.append((0, qc, sub))
            # b1 outproj units become ready progressively; mix them in so
            # the final phase is short.
            ready_b1 = deque()

            def pop_units(k):
                for _ in range(k):
                    if ready_b1:
                        outproj_unit(*ready_b1.popleft())
                    elif ofill:
                        outproj_unit(*ofill.popleft())

            pend = deque()
            for qc in range(NQ):
                for h in range(HL):
                    emit_scores(1, qc, h)
                    pop_units(3)
                    if len(pend) >= 2:
                        g = pend.popleft()
                        emit_tail(*g)
                        if g[2] == 1:
                            # g's qc now has OT complete for both heads
                            for sub in range(4):
                                ready_b1.append((1, g[1], sub))
                    pend.append((1, qc, h))
                    if h == 1 and qc + 1 < NQ:
                        issue_masks(1, qc + 1)
                pop_units(1)
            while pend:
                g = pend.popleft()
                emit_tail(*g)
                pop_units(2)
            for sub in range(4):
                ready_b1.append((1, NQ - 1, sub))

            # --- phase D: drain remaining outproj units (fine-grained
            # output DMA so the last transfer is small) ---
            while ofill:
                outproj_unit(*ofill.popleft(), fine=True)
            while ready_b1:
                outproj_unit(*ready_b1.popleft(), fine=True)

    nc.compile()
    return nc


def get_nc(nktts):
    key = tuple(nktts)
    if key not in _CACHE:
        _CACHE[key] = _build(key)
    return _CACHE[key]


def make_in_maps(q, k, v, Wq, Wk, Wv, Wo, attn_mask, key_padding_mask):
    scale = np.float32(1.0 / np.sqrt(np.float32(DH)))

    qT = q.reshape(BT, D).T.astype(BF16)
    qTc = np.ascontiguousarray(
        qT.reshape(KC, 128, NBT, 512).transpose(2, 1, 0, 3)
        .reshape(NBT, 128, KC * 512))

    # ---- compact the key dimension: drop fully-padded keys ----
    kpm = np.asarray(key_padding_mask)
    idx = [np.nonzero(~kpm[b])[0] for b in range(B)]
    nktts = [max(2, -(-len(ix) // 128)) for ix in idx]
    ncolsb = [n * 128 for n in nktts]
    kC = [np.zeros((nc_, D), np.float32) for nc_ in ncolsb]
    vC = [np.zeros((nc_, D), np.float32) for nc_ in ncolsb]
    for b in range(B):
        kC[b][:len(idx[b])] = k[b, idx[b]]
        vC[b][:len(idx[b])] = v[b, idx[b]]

    def flat_chunks(xC):
        blocks = []
        for b in range(B):
            ncols = ncolsb[b]
            xT = xC[b].T.astype(BF16)  # [D, ncols]
            a = xT.reshape(KC, 128, ncols)
            for j0, w in _kv_chunks(ncols):
                # block [128, KC, w], p-major
                blocks.append(np.ascontiguousarray(
                    a[:, :, j0:j0 + w].transpose(1, 0, 2)).reshape(-1))
        return np.ascontiguousarray(np.concatenate(blocks))

    kTc = flat_chunks(kC)
    vTc = flat_chunks(vC)

    # compacted multiplicative transposed mask, flat [(b), qc, kt, p, j]
    mblocks = []
    for b in range(B):
        ncols = ncolsb[b]
        mCb = np.zeros((ncols, T), np.float32)
        mCb[:len(idx[b])] = (~attn_mask[:, idx[b]].T).astype(np.float32)
        mtb = (mCb.astype(BF16)
               .reshape(nktts[b], 128, NQ, 512)
               .transpose(2, 0, 1, 3))      # [NQ, kt, 128, 512]
        mblocks.append(np.ascontiguousarray(mtb).reshape(-1))
    maskt = np.ascontiguousarray(np.concatenate(mblocks))

    def prep_w(wT):  # [D, DL] -> [128, KC*DL]
        return np.ascontiguousarray(
            wT.reshape(KC, 128, DL).transpose(1, 0, 2).reshape(128, KC * DL)
            .astype(BF16))

    in_maps = []
    for i in range(N_CORES):
        rows = slice(i * DL, (i + 1) * DL)
        wq_i = prep_w(Wq[rows, :].T * scale)
        wk_i = prep_w(Wk[rows, :].T)
        wv_i = prep_w(Wv[rows, :].T)
        woT = Wo[:, rows].T  # [DL, D]
        wo_i = np.ascontiguousarray(
            woT.reshape(HL, 128, D).transpose(1, 0, 2).reshape(128, HL * D)
            .astype(BF16))
        in_maps.append({
            "qTc": qTc, "kTc": kTc, "vTc": vTc,
            "wq": wq_i, "wk": wk_i, "wv": wv_i, "wo": wo_i,
            "maskt": maskt,
        })
    return in_maps, nktts


def postprocess(results):
    acc = np.zeros((BT, D), np.float32)
    for r in results:
        acc += r["out"].astype(np.float32)
    return acc.reshape(B, T, D)


def kernel(**inputs):
    inputs = {k: np.asarray(v) for k, v in inputs.items()}
    in_maps, nktts = make_in_maps(**inputs)
    nc = get_nc(nktts)
    res = bass_utils.run_bass_kernel_spmd(
        nc, in_maps, core_ids=list(range(N_CORES)))
    return postprocess(res.results)
